# revision 10
# baseline (speedup 1.0000x reference)
"""LlamaMoE (H=2048, I=4096, E=8 experts, top-2, N=2048 tokens) on 8 trn2 cores.

Strategy: PAIR-SPLIT expert parallelism + token-parallel base MLP, combined
with a single split AllToAll.

The tensor engine is power-throttled to ~1.9 GHz sustained and the baseline
schedule had zero tensor idle, so the only win is fewer matmul cycles.
Expert token counts are imbalanced (484..545 vs 512 avg); with one expert
per core every core pays for the worst count. Instead, experts are PAIRED
large-with-small (sums 1019..1029, +-0.5%): the two cores of a pair each
hold HALF of the intermediate dim (I/2) of BOTH experts and process the
pair's full token list. Each (token, expert) down-projection row is then a
half-I partial computed on two cores; both partials ride the AllToAll and
the home core sums 4 contributions (2 experts x 2 halves) instead of 2.
Per-core matmul cycles drop ~6% and are balanced regardless of routing
skew, with identical weight DMA volume.

Host supplies the dispatch permutation (pre-gathered transposed activations
plus send/receive index maps padded with OOB sentinels); all model math --
router logits, top-2 combine weights, expert MLPs, base MLP, combine --
runs on device. The base MLP stays row-sharded: core c computes the full
base MLP for its own 256 token rows (no cross-core reduction).

Expert partial rows are scattered into an AllToAll send buffer grouped by
destination (token-home) core; one fp16 AllToAll per H-quarter fires as
soon as that column chunk of the down-projection completes, so all four
collectives drain during mm2e itself (light DMA phase) instead of starving
mm1b's weight stream. The home core computes the router (fp32) on its own tokens for the
top-2 combine weights, accumulates the 4 weighted contribution rows per
token (no base dependency), adds the base rows when they land, and writes
its 256-row output shard.
"""

import numpy as np

import concourse.bacc as bacc
import concourse.bass as bass
import concourse.mybir as mybir
import concourse.tile as tile
from concourse.bass_utils import run_bass_kernel_spmd

P = 128
H = 2048
I_EXP = 4096
HI = I_EXP // 2             # half intermediate dim per core
E = 8
NCORE = 8
NPAIR = NCORE // 2
NTOK = 2048
TOWN = NTOK // NCORE        # 256 own token rows per core
TOB = TOWN // P             # 2 own token blocks
KO = H // P                 # 16 contraction tiles for mm1
SLE = HI // P               # 16 half-I slabs per expert section
IC_B = I_EXP // P           # 32 base chunks (full I, row-sharded base)
ICT = 2 * SLE + IC_B        # 64 gate/up slabs (a-half, b-half, base)
NB1 = 512                   # mm1 expert moving free dim (tokens)
HN = 512                    # mm2 moving free dim (H cols) = A2A quarter width
HNC = H // HN               # 4
HF = H // 2                 # column half (base mm2 lo/hi split)
NCON = 4                    # combine contributions per token (2 exp x 2 half)

F32 = mybir.dt.float32
F16 = mybir.dt.float16
I32 = mybir.dt.int32
AF = mybir.ActivationFunctionType
ALU = mybir.AluOpType
AXX = mybir.AxisListType.X

OOB_IDX = 1 << 20


def _chunks(total, step):
    out = []
    o = 0
    while o < total:
        out.append((o, min(step, total - o)))
        o += step
    return out


def _build(CA, CB, SLOT):
    NTCA = (CA + P - 1) // P
    NTCB = (CB + P - 1) // P
    NTC = NTCA + NTCB
    CAp, CBp = NTCA * P, NTCB * P
    nc = bacc.Bacc(None)
    xeT_d = nc.dram_tensor("xeT", [P, KO, CAp + CBp], F16, kind="ExternalInput")
    xtO_d = nc.dram_tensor("xtO", [P, KO, TOWN], F16, kind="ExternalInput")
    xrO_d = nc.dram_tensor("xrO", [P, KO, TOWN], F32, kind="ExternalInput")
    wgu_d = nc.dram_tensor("wgu", [P, ICT, KO, 2 * P], F16, kind="ExternalInput")
    wde_d = nc.dram_tensor("wde", [P, 2, HNC, SLE, HN], F16, kind="ExternalInput")
    wdb_d = nc.dram_tensor("wdb", [P, HNC, IC_B, HN], F16, kind="ExternalInput")
    gw_d = nc.dram_tensor("gw", [P, KO, E], F32, kind="ExternalInput")
    dsti_d = nc.dram_tensor("dsti", [P, NTC], I32, kind="ExternalInput")
    rvi_d = nc.dram_tensor("rvi", [P, NCON * TOB], I32, kind="ExternalInput")
    cbi_d = nc.dram_tensor("cbi", [P, NCON * TOB], I32, kind="ExternalInput")
    out_d = nc.dram_tensor("out", [TOB, P, H], F16, kind="ExternalOutput")

    from contextlib import ExitStack
    with tile.TileContext(nc) as tc:
        with ExitStack() as _stk:
            def _pool(**kw):
                return _stk.enter_context(tc.tile_pool(**kw))
            persist = _pool(name="persist", bufs=1)
            xtp = _pool(name="xt", bufs=1)
            htp = _pool(name="ht", bufs=1)
            htbp = _pool(name="htb", bufs=1)
            xtop = _pool(name="xto", bufs=1)
            wgup = _pool(name="wgup", bufs=4)
            wdp = _pool(name="wdp", bufs=4)
            xk32p = _pool(name="xk32", bufs=1)
            tmpp = _pool(name="tmp", bufs=2)
            yesp = _pool(name="yesp", bufs=2)
            rgp = _pool(name="rgp", bufs=3)
            osbp = _pool(name="osb", bufs=1)
            rsm = _pool(name="rsm", bufs=1)
            ps1 = _pool(name="ps1", bufs=2, space="PSUM")
            ps2 = _pool(name="ps2", bufs=2, space="PSUM")
            psr = _pool(name="psr", bufs=1, space="PSUM")
            dram = _pool(name="dram", bufs=1, space="DRAM")
            send_dram = [
                dram.tile([NCORE * SLOT, HN], F16, tag=f"send{q_}", name=f"send{q_}")
                for q_ in range(HNC)
            ]
            recv_dram = [
                dram.tile([NCORE * SLOT, HN], F16, tag=f"recv{q_}", name=f"recv{q_}")
                for q_ in range(HNC)
            ]
            comb_dram = dram.tile([TOWN * E, 1], F32, tag="combd")

            # ===== mm1 expert: gate/up + silu*up on both half-experts ======
            # xeT columns: [pair-expert-a tokens | pad | expert-b tokens |
            # pad]; pads are zero so h comes out 0 and the rows map to OOB
            # send slots downstream. xeT arrives in k-chunks so the first
            # matmuls start as soon as chunk 0 lands.
            xeT = xtp.tile([P, KO, CAp + CBp], F16, tag="xt", name="xeT")
            for kq in range(4):
                nc.sync.dma_start(
                    xeT[:, kq * (KO // 4):(kq + 1) * (KO // 4), :],
                    xeT_d[:, kq * (KO // 4):(kq + 1) * (KO // 4)],
                )
            gw_sb = persist.tile([P, KO, E], F32, tag="gw")
            nc.sync.dma_start(gw_sb, gw_d[:])
            dsti_sb = persist.tile([P, NTC], I32, tag="dsti")
            nc.sync.dma_start(dsti_sb, dsti_d[:])
            rvi_sb = persist.tile([P, NCON * TOB], I32, tag="rvi")
            nc.sync.dma_start(rvi_sb, rvi_d[:])
            cbi_sb = persist.tile([P, NCON * TOB], I32, tag="cbi")
            nc.sync.dma_start(cbi_sb, cbi_d[:])
            ht_e = htp.tile([P, SLE, CAp + CBp], F16, tag="hte")
            for sect in range(2):
                Cs = CA if sect == 0 else CB
                coff = 0 if sect == 0 else CAp
                for i in range(SLE):
                    slab = wgup.tile(
                        [P, KO, 2 * P], F16, tag="slab", name=f"sl{sect}_{i}"
                    )
                    nc.sync.dma_start(slab, wgu_d[:, sect * SLE + i])
                    for (no, nw) in _chunks(Cs, NB1):
                        nsl = slice(coff + no, coff + no + nw)
                        pg = ps1.tile([P, NB1], F32, tag="pg", name=f"pg{sect}_{i}_{no}")
                        pu = ps1.tile([P, NB1], F32, tag="pu", name=f"pu{sect}_{i}_{no}")
                        for k in range(KO):
                            nc.tensor.matmul(
                                pg[:, :nw], slab[:, k, 0:P], xeT[:, k, nsl],
                                start=(k == 0), stop=(k == KO - 1),
                            )
                        for k in range(KO):
                            nc.tensor.matmul(
                                pu[:, :nw], slab[:, k, P:2 * P], xeT[:, k, nsl],
                                start=(k == 0), stop=(k == KO - 1),
                            )
                        sil = tmpp.tile([P, NB1], F16, tag="sil")
                        nc.scalar.activation(sil[:, :nw], pg[:, :nw], AF.Silu)
                        nc.vector.tensor_tensor(
                            ht_e[:, i, nsl], sil[:, :nw], pu[:, :nw], ALU.mult
                        )

            # ============ mm1 base: own 256 tokens, full I =================
            # Runs BEFORE mm2e: mm1b streams weights at ~240 GB/s and must
            # not share the HBM with AllToAll drains; the collectives fire
            # during mm2e/mm2b below, whose DMA load is light.
            xtO = xtop.tile([P, KO, TOWN], F16, tag="xto", name="xtO")
            nc.sync.dma_start(xtO, xtO_d[:])
            ht_b = htbp.tile([P, IC_B, TOWN], F16, tag="htb", name="ht_b")
            for j in range(IC_B):
                slab = wgup.tile([P, KO, 2 * P], F16, tag="slab", name=f"slb{j}")
                slab_dma = nc.sync.dma_start(slab, wgu_d[:, 2 * SLE + j])
                if j == IC_B - 1:
                    last_mm1b_slab_dma = slab_dma
                pg = ps1.tile([P, TOWN], F32, tag="pg", name=f"bpg{j}")
                pu = ps1.tile([P, TOWN], F32, tag="pu", name=f"bpu{j}")
                # interleave gate/up so each LDWEIGHTS hides under the
                # previous matmul (N=256 leaves no slack otherwise)
                for k in range(KO):
                    nc.tensor.matmul(
                        pg, slab[:, k, 0:P], xtO[:, k, :],
                        start=(k == 0), stop=(k == KO - 1),
                    )
                    nc.tensor.matmul(
                        pu, slab[:, k, P:2 * P], xtO[:, k, :],
                        start=(k == 0), stop=(k == KO - 1),
                    )
                sil = tmpp.tile([P, TOWN], F16, tag="sil")
                nc.scalar.activation(sil, pg, AF.Silu)
                nc.vector.tensor_tensor(ht_b[:, j, :], sil, pu, ALU.mult)

            # ===== mm2 expert (half-I down partials) on dispatched tokens ==
            # One H-quarter per cc chunk: scatter each block's rows as they
            # finish and fire that quarter's AllToAll immediately, so all
            # collective traffic drains during mm2e (light DMA load) instead
            # of colliding with mm1b's weight streaming.
            for cc in range(HNC):
                subs = []
                for sect in range(2):
                    ss = []
                    for sub in range(2):
                        w = wdp.tile(
                            [P, SLE // 2, HN], F16, tag="wsl",
                            name=f"we{cc}_{sect}_{sub}",
                        )
                        nc.sync.dma_start(
                            w,
                            wde_d[:, sect, cc,
                                  sub * (SLE // 2):(sub + 1) * (SLE // 2)],
                        )
                        ss.append(w)
                    subs.append(ss)
                yesq = yesp.tile([P, NTC, HN], F16, tag="yes", name=f"yes{cc}")
                for bi in range(NTC):
                    sect = 0 if bi < NTCA else 1
                    col = bi * P if sect == 0 else CAp + (bi - NTCA) * P
                    py = ps2.tile([P, HN], F32, tag="py", name=f"pye{cc}_{bi}")
                    for k in range(SLE):
                        nc.tensor.matmul(
                            py, ht_e[:, k, col:col + P],
                            subs[sect][k // (SLE // 2)][:, k % (SLE // 2), :],
                            start=(k == 0), stop=(k == SLE - 1),
                        )
                    nc.scalar.activation(yesq[:, bi, :], py, AF.Copy)
                    nc.gpsimd.indirect_dma_start(
                        out=send_dram[cc][:],
                        out_offset=bass.IndirectOffsetOnAxis(
                            ap=dsti_sb[:, bi:bi + 1], axis=0
                        ),
                        in_=yesq[:, bi, :],
                        in_offset=None,
                        bounds_check=NCORE * SLOT - 1,
                        oob_is_err=False,
                    )
                nc.gpsimd.collective_compute(
                    "AllToAll",
                    ALU.bypass,
                    replica_groups=[list(range(NCORE))],
                    ins=[send_dram[cc][:]],
                    outs=[recv_dram[cc][:]],
                )

            # ============ router on own 256 tokens (strict fp32) ===========
            # logits^T: stationary = own x^T block [128h, 128tok], moving =
            # gw [128h, 8]; accumulate over k. One accumulation group at a
            # time per PSUM bank (start=True clears the whole bank's bits).
            zl_ps = psr.tile([P, TOB, E], F32, tag="zlps")
            for tb in range(TOB):
                xk = xk32p.tile([P, KO, P], F32, tag="xk")
                nc.sync.dma_start(xk, xrO_d[:, :, tb * P:(tb + 1) * P])
                for k in range(KO):
                    nc.tensor.matmul(
                        zl_ps[:, tb, :],
                        xk[:, k, :],
                        gw_sb[:, k, :],
                        start=(k == 0), stop=(k == KO - 1),
                    )
            zl = rsm.tile([P, TOB, E], F32, tag="zl")
            nc.vector.tensor_copy(zl, zl_ps)
            lmax = rsm.tile([P, TOB], F32, tag="lmax")
            nc.vector.reduce_max(lmax[:, :, None], zl, axis=AXX)
            nmax = rsm.tile([P, TOB], F32, tag="nmax")
            nc.vector.tensor_scalar_mul(nmax, lmax, -1.0)
            zex = rsm.tile([P, TOB, E], F32, tag="zex")
            for tb in range(TOB):
                nc.scalar.activation(
                    zex[:, tb, :], zl[:, tb, :], AF.Exp, bias=nmax[:, tb:tb + 1]
                )
            zlt = rsm.tile([P, TOB, E], F32, tag="zlt")
            nc.vector.tensor_scalar(zlt, zex, 1.0, None, op0=ALU.is_lt)
            zmk = rsm.tile([P, TOB, E], F32, tag="zmk")
            nc.vector.tensor_tensor(zmk, zex, zlt, ALU.mult)
            m2 = rsm.tile([P, TOB], F32, tag="m2")
            nc.vector.reduce_max(m2[:, :, None], zmk, axis=AXX)
            # per-expert top-2 mask and normalized weights: w_e =
            # zex_e * [zex_e >= m2] / (1 + m2)
            ge = rsm.tile([P, TOB, E], F32, tag="ge")
            nc.vector.tensor_tensor(
                ge, zex, m2[:, :, None].to_broadcast((P, TOB, E)), ALU.is_ge
            )
            s1 = rsm.tile([P, TOB], F32, tag="s1")
            nc.vector.tensor_scalar_add(s1, m2, 1.0)
            rcp = rsm.tile([P, TOB], F32, tag="rcp")
            nc.vector.reciprocal(rcp, s1)
            cw = rsm.tile([P, TOB, E], F32, tag="cw")
            nc.vector.tensor_tensor(cw, zex, ge, ALU.mult)
            cwn = rsm.tile([P, TOB, E], F32, tag="cwn")
            nc.vector.tensor_tensor(
                cwn, cw, rcp[:, :, None].to_broadcast((P, TOB, E)), ALU.mult
            )
            # store [TOWN*E, 1] with flat index (tb*128 + p)*8 + e
            nc.sync.dma_start(
                comb_dram[:].rearrange(
                    "(b p e) one -> p b (e one)", p=P, b=TOB, e=E
                ),
                cwn,
            )

            # prefetch combine-weight rows (router output, ready long ago)
            # before the gpsimd queue blocks on the collective
            cbs = []
            for sidx in range(NCON * TOB):
                cb = rgp.tile([P, 1], F32, tag=f"cb{sidx}", name=f"cb{sidx}")
                nc.gpsimd.indirect_dma_start(
                    out=cb[:],
                    out_offset=None,
                    in_=comb_dram[:],
                    in_offset=bass.IndirectOffsetOnAxis(
                        ap=cbi_sb[:, sidx:sidx + 1], axis=0
                    ),
                    bounds_check=TOWN * E - 1,
                    oob_is_err=False,
                )
                cbs.append(cb)
            # ===== receive: gather 4 partial rows per token, accumulate ====
            # The weighted expert accumulation has no base dependency, so it
            # runs on the vector engine underneath the base down-projection.
            accs = [
                osbp.tile([P, H], F16, tag=f"osb{tb_}", name=f"osb{tb_}")
                for tb_ in range(TOB)
            ]
            for q in range(HNC):
                qsl = slice(q * HN, (q + 1) * HN)
                for tb in range(TOB):
                    for j in range(NCON):
                        sidx = j * TOB + tb
                        rg = rgp.tile([P, HN], F16, tag="rg")
                        rg_dma = nc.gpsimd.indirect_dma_start(
                            out=rg[:],
                            out_offset=None,
                            in_=recv_dram[q][:],
                            in_offset=bass.IndirectOffsetOnAxis(
                                ap=rvi_sb[:, sidx:sidx + 1], axis=0
                            ),
                            bounds_check=NCORE * SLOT - 1,
                            oob_is_err=False,
                        )
                        # Pin behind mm1b's last weight load: issued earlier,
                        # this gather's A2A-completion wait head-of-line
                        # blocks later DMAs sharing its completion lane.
                        bass._add_dep_helper(
                            rg_dma.ins, last_mm1b_slab_dma.ins, sync=True,
                            reason="defer recv gather",
                        )
                        if j == 0:
                            nc.vector.tensor_scalar_mul(
                                accs[tb][:, qsl], rg[:], cbs[sidx][:]
                            )
                        else:
                            nc.vector.scalar_tensor_tensor(
                                accs[tb][:, qsl], rg[:], cbs[sidx][:],
                                accs[tb][:, qsl], ALU.mult, ALU.add,
                            )

            # ============ mm2 base (down) on own tokens ====================
            base_lo = yesp.tile([P, TOB, HF], F16, tag="yes", name="base_lo")
            base_hi = xk32p.tile([P, TOB, HF], F16, tag="xk", name="base_hi")
            for cc in range(HNC):
                nsub = IC_B // (SLE // 2)
                bsubs = []
                for ss in range(nsub):
                    w = wdp.tile(
                        [P, SLE // 2, HN], F16, tag="wsl", name=f"wb{cc}_{ss}"
                    )
                    nc.sync.dma_start(
                        w, wdb_d[:, cc, ss * (SLE // 2):(ss + 1) * (SLE // 2)]
                    )
                    bsubs.append(w)
                for tb in range(TOB):
                    py = ps2.tile([P, HN], F32, tag="py", name=f"pyb{cc}_{tb}")
                    for j in range(IC_B):
                        nc.tensor.matmul(
                            py, ht_b[:, j, tb * P:(tb + 1) * P],
                            bsubs[j // (SLE // 2)][:, j % (SLE // 2), :],
                            start=(j == 0), stop=(j == IC_B - 1),
                        )
                    bdst = base_lo if cc < HNC // 2 else base_hi
                    bcc = cc % (HNC // 2)
                    nc.scalar.activation(
                        bdst[:, tb, bcc * HN:(bcc + 1) * HN], py, AF.Copy
                    )

            # ====== add base rows; low half first so it hides under the ====
            # ====== remaining base down-projection; write output shard =====
            for half in range(2):
                hsl = slice(half * HF, (half + 1) * HF)
                base_h = (base_lo, base_hi)[half]
                for tb in range(TOB):
                    nc.vector.tensor_tensor(
                        accs[tb][:, hsl], accs[tb][:, hsl], base_h[:, tb, :],
                        ALU.add,
                    )
            for tb in range(TOB):
                nc.sync.dma_start(out_d[tb], accs[tb])

    return nc


def _prep_inputs(x, gate_w, base_gate_up, base_down, expert_gate_up, expert_down):
    xf = np.ascontiguousarray(np.asarray(x, np.float32).reshape(NTOK, H))
    xT = np.ascontiguousarray(xf.reshape(NTOK, KO, P).transpose(2, 1, 0))
    xt16 = xT.astype(np.float16)
    gwf = np.asarray(gate_w, np.float32)
    gwp = np.ascontiguousarray(gwf.reshape(KO, P, E).transpose(1, 0, 2))

    # host-side dispatch: which tokens go to which expert (top-2 of logits)
    logits = xf @ gwf
    order = np.argsort(-logits, axis=1)
    top2 = order[:, :2]
    sel = [np.where((top2 == c).any(axis=1))[0].astype(np.int64) for c in range(E)]
    counts = np.array([len(s) for s in sel])

    # pair heavy experts with light ones so pair token sums are near-equal;
    # cores 2p / 2p+1 hold the low / high I-halves of pair p's two experts
    od = np.argsort(-counts, kind="stable")
    pairs = [(int(od[i]), int(od[E - 1 - i])) for i in range(NPAIR)]
    e2pr = {}
    for pi, (a, b) in enumerate(pairs):
        e2pr[a] = (pi, 0)
        e2pr[b] = (pi, 1)

    CA = int(max(counts[a] for a, b in pairs))
    CB = int(max(counts[b] for a, b in pairs))
    NTCA = (CA + P - 1) // P
    NTCB = (CB + P - 1) // P
    CAp, CBp = NTCA * P, NTCB * P
    NTC = NTCA + NTCB

    # per-pair concatenated token columns: [a tokens|pad] + [b tokens|pad];
    # send position = order of appearance within the (pair -> home) group
    pair_flat = []
    pair_pos = []
    pair_colof = []
    max_grp = 0
    for pi, (a, b) in enumerate(pairs):
        La, Lb = sel[a], sel[b]
        flat = np.full(CAp + CBp, -1, np.int64)
        flat[: len(La)] = La
        flat[CAp:CAp + len(Lb)] = Lb
        real = np.nonzero(flat >= 0)[0]
        pos = np.full(CAp + CBp, OOB_IDX, np.int64)
        cnt = np.zeros(NCORE, np.int64)
        colof = {}
        for ci in real:
            hm = flat[ci] // TOWN
            pos[ci] = cnt[hm]
            cnt[hm] += 1
            colof[(int(flat[ci]), 0 if ci < CAp else 1)] = ci
        max_grp = max(max_grp, int(cnt.max()))
        pair_flat.append(flat)
        pair_pos.append(pos)
        pair_colof.append(colof)
    SLOT = (max_grp + 3) // 4 * 4

    # per-pair send index: column (block bi, partition p) -> home*SLOT + pos
    dsti_p = []
    for pi in range(NPAIR):
        flat, pos = pair_flat[pi], pair_pos[pi]
        dst = np.where(
            flat >= 0, (flat // TOWN) * SLOT + pos, OOB_IDX
        ).astype(np.int64)
        dsti_p.append(
            np.ascontiguousarray(dst.reshape(NTC, P).T.astype(np.int32))
        )

    # per-core receive index: own token t, contribution j in 0..3 =
    # (expert rank j//2 sorted, I-half j%2): recv row = sender*SLOT + pos
    rvi = np.zeros((NCORE, P, NCON * TOB), np.int32)
    cbi = np.zeros((NCORE, P, NCON * TOB), np.int32)
    for hme in range(NCORE):
        for tl in range(TOWN):
            t = hme * TOWN + tl
            tb, p = divmod(tl, P)
            es = np.sort(top2[t])
            for j2, e in enumerate(es):
                pi, role = e2pr[int(e)]
                ci = pair_colof[pi][(t, role)]
                ps_ = int(pair_pos[pi][ci])
                for half in range(2):
                    j = j2 * 2 + half
                    sender = 2 * pi + half
                    rvi[hme, p, j * TOB + tb] = sender * SLOT + ps_
                    cbi[hme, p, j * TOB + tb] = tl * E + e

    bgu = np.asarray(base_gate_up, np.float32)
    gb_ = bgu[:, :I_EXP].reshape(H, IC_B, P)
    ub_ = bgu[:, I_EXP:].reshape(H, IC_B, P)
    pb_ = np.concatenate([gb_, ub_], axis=2)  # [H, IC_B, 2P]
    bd = np.asarray(base_down, np.float32)
    wdb_p = np.ascontiguousarray(
        bd.reshape(IC_B, P, HNC, HN).transpose(1, 2, 0, 3)
    ).astype(np.float16)

    in_maps = []
    for c in range(NCORE):
        pi, half = divmod(c, 2)
        a, b = pairs[pi]

        def half_slabs(We):
            We = np.asarray(We, np.float32)
            g = We[:, half * HI:(half + 1) * HI].reshape(H, SLE, P)
            u = We[:, I_EXP + half * HI:I_EXP + (half + 1) * HI].reshape(
                H, SLE, P
            )
            return np.concatenate([g, u], axis=2)  # [H, SLE, 2P]

        allp = np.concatenate(
            [half_slabs(expert_gate_up[a]), half_slabs(expert_gate_up[b]), pb_],
            axis=1,
        )  # [H, ICT, 2P]
        wgu_p = np.ascontiguousarray(
            allp.reshape(KO, P, ICT, 2 * P).transpose(1, 2, 0, 3)
        ).astype(np.float16)

        def half_down(ed_):
            d = np.asarray(ed_, np.float32)[half * HI:(half + 1) * HI]
            return d.reshape(SLE, P, HNC, HN).transpose(1, 2, 0, 3)

        wde_p = np.ascontiguousarray(
            np.stack(
                [half_down(expert_down[a]), half_down(expert_down[b])], axis=1
            )
        ).astype(np.float16)  # [P, 2, HNC, SLE, HN]

        flat = pair_flat[pi]
        real = flat >= 0
        xe = np.zeros((P, KO, CAp + CBp), np.float16)
        xe[:, :, real] = xt16[:, :, flat[real]]
        own = slice(c * TOWN, (c + 1) * TOWN)
        in_maps.append(
            dict(
                xeT=np.ascontiguousarray(xe),
                xtO=np.ascontiguousarray(xt16[:, :, own]),
                xrO=np.ascontiguousarray(xT[:, :, own]),
                wgu=wgu_p, wde=wde_p, wdb=wdb_p, gw=gwp,
                dsti=dsti_p[pi], rvi=rvi[c], cbi=cbi[c],
            )
        )
    return in_maps, CA, CB, SLOT


LAST_RESULTS = None


def kernel(x, gate_w, base_gate_up, base_down, expert_gate_up, expert_down):
    global LAST_RESULTS
    in_maps, CA, CB, SLOT = _prep_inputs(
        x, gate_w, base_gate_up, base_down, expert_gate_up, expert_down
    )
    nc = _build(CA, CB, SLOT)
    if not nc.is_finalized():
        nc.finalize()
    res = run_bass_kernel_spmd(nc, in_maps, core_ids=list(range(NCORE)))
    LAST_RESULTS = res
    y = np.empty((NTOK, H), np.float32)
    for c in range(NCORE):
        o = res.results[c]["out"]  # [TOB, P, H] f16
        y[c * TOWN:(c + 1) * TOWN] = o.reshape(TOWN, H).astype(np.float32)
    return y.reshape(1, NTOK, H)


if __name__ == "__main__":
    nc = _build(545, 510, 160)
    print("build ok; instructions:",
          sum(len(b.instructions) for b in nc.main_func.blocks))


# revision 11
# speedup vs baseline: 1.0766x; 1.0766x over previous
"""LlamaMoE (H=2048, I=4096, E=8 experts, top-2, N=2048 tokens) on 8 trn2 cores.

Strategy: PAIR-SPLIT expert parallelism + token-parallel base MLP, combined
with a single split AllToAll.

The tensor engine is power-throttled to ~1.9 GHz sustained and the baseline
schedule had zero tensor idle, so the only win is fewer matmul cycles.
Expert token counts are imbalanced (484..545 vs 512 avg); with one expert
per core every core pays for the worst count. Instead, experts are PAIRED
large-with-small (sums 1019..1029, +-0.5%): the two cores of a pair each
hold HALF of the intermediate dim (I/2) of BOTH experts and process the
pair's full token list. Each (token, expert) down-projection row is then a
half-I partial computed on two cores; both partials ride the AllToAll and
the home core sums 4 contributions (2 experts x 2 halves) instead of 2.
Per-core matmul cycles drop ~6% and are balanced regardless of routing
skew, with identical weight DMA volume.

Host supplies the dispatch permutation (pre-gathered transposed activations
plus send/receive index maps padded with OOB sentinels); all model math --
router logits, top-2 combine weights, expert MLPs, base MLP, combine --
runs on device. The base MLP stays row-sharded: core c computes the full
base MLP for its own 256 token rows (no cross-core reduction).

Expert partial rows are scattered into an AllToAll send buffer grouped by
destination (token-home) core; one fp16 AllToAll per H-quarter fires as
soon as that column chunk of the down-projection completes, so all four
collectives drain during mm2e itself (light DMA phase) instead of starving
mm1b's weight stream. The home core computes the router (fp32) on its own tokens for the
top-2 combine weights, accumulates the 4 weighted contribution rows per
token (no base dependency), adds the base rows when they land, and writes
its 256-row output shard.
"""

import numpy as np

import concourse.bacc as bacc
import concourse.bass as bass
import concourse.mybir as mybir
import concourse.tile as tile
from concourse.bass_utils import run_bass_kernel_spmd

P = 128
H = 2048
I_EXP = 4096
HI = I_EXP // 2             # half intermediate dim per core
E = 8
NCORE = 8
NPAIR = NCORE // 2
NTOK = 2048
TOWN = NTOK // NCORE        # 256 own token rows per core
TOB = TOWN // P             # 2 own token blocks
KO = H // P                 # 16 contraction tiles for mm1
SLE = HI // P               # 16 half-I slabs per expert section
IC_B = I_EXP // P           # 32 base chunks (full I, row-sharded base)
ICT = 2 * SLE + IC_B        # 64 gate/up slabs (a-half, b-half, base)
NB1 = 512                   # mm1 expert moving free dim (tokens)
HN = 512                    # mm2 moving free dim (H cols) = A2A quarter width
HNC = H // HN               # 4
HF = H // 2                 # column half (base mm2 lo/hi split)
NCON = 4                    # combine contributions per token (2 exp x 2 half)

F32 = mybir.dt.float32
F16 = mybir.dt.float16
I32 = mybir.dt.int32
AF = mybir.ActivationFunctionType
ALU = mybir.AluOpType
AXX = mybir.AxisListType.X

OOB_IDX = 1 << 20


def _chunks(total, step):
    out = []
    o = 0
    while o < total:
        out.append((o, min(step, total - o)))
        o += step
    return out


def _build(CA, CB, SLOT):
    NTCA = (CA + P - 1) // P
    NTCB = (CB + P - 1) // P
    NTC = NTCA + NTCB
    CAp, CBp = NTCA * P, NTCB * P
    nc = bacc.Bacc(None)
    xeT_d = nc.dram_tensor("xeT", [P, KO, CAp + CBp], F16, kind="ExternalInput")
    xtO_d = nc.dram_tensor("xtO", [P, KO, TOWN], F16, kind="ExternalInput")
    xrO_d = nc.dram_tensor("xrO", [P, KO, TOWN], F32, kind="ExternalInput")
    wgu_d = nc.dram_tensor("wgu", [P, ICT, KO, 2 * P], F16, kind="ExternalInput")
    wde_d = nc.dram_tensor("wde", [P, 2, HNC, SLE, HN], F16, kind="ExternalInput")
    wdb_d = nc.dram_tensor("wdb", [P, HNC, IC_B, HN], F16, kind="ExternalInput")
    gw_d = nc.dram_tensor("gw", [P, KO, E], F32, kind="ExternalInput")
    dsti_d = nc.dram_tensor("dsti", [P, NTC], I32, kind="ExternalInput")
    rvi_d = nc.dram_tensor("rvi", [P, NCON * TOB], I32, kind="ExternalInput")
    cbi_d = nc.dram_tensor("cbi", [P, NCON * TOB], I32, kind="ExternalInput")
    out_d = nc.dram_tensor("out", [TOB, P, H], F16, kind="ExternalOutput")

    from contextlib import ExitStack
    with tile.TileContext(nc) as tc:
        with ExitStack() as _stk:
            def _pool(**kw):
                return _stk.enter_context(tc.tile_pool(**kw))
            persist = _pool(name="persist", bufs=1)
            xtp = _pool(name="xt", bufs=1)
            htp = _pool(name="ht", bufs=1)
            wgup = _pool(name="wgup", bufs=6)
            wdp = _pool(name="wdp", bufs=4)
            xk32p = _pool(name="xk32", bufs=1)
            tmpp = _pool(name="tmp", bufs=2)
            yesp = _pool(name="yesp", bufs=2)
            rgp = _pool(name="rgp", bufs=3)
            osbp = _pool(name="osb", bufs=1)
            rsm = _pool(name="rsm", bufs=1)
            ps1 = _pool(name="ps1", bufs=2, space="PSUM")
            ps2 = _pool(name="ps2", bufs=2, space="PSUM")
            psr = _pool(name="psr", bufs=1, space="PSUM")
            dram = _pool(name="dram", bufs=1, space="DRAM")
            send_dram = [
                dram.tile([NCORE * SLOT, HN], F16, tag=f"send{q_}", name=f"send{q_}")
                for q_ in range(HNC)
            ]
            recv_dram = [
                dram.tile([NCORE * SLOT, HN], F16, tag=f"recv{q_}", name=f"recv{q_}")
                for q_ in range(HNC)
            ]
            comb_dram = dram.tile([TOWN * E, 1], F32, tag="combd")

            # ===== mm1 expert: gate/up + silu*up on both half-experts ======
            # xeT columns: [pair-expert-a tokens | pad | expert-b tokens |
            # pad]; pads are zero so h comes out 0 and the rows map to OOB
            # send slots downstream. xeT arrives in k-chunks so the first
            # matmuls start as soon as chunk 0 lands.
            xeT = xtp.tile([P, KO, CAp + CBp], F16, tag="xt", name="xeT")
            for kq in range(4):
                nc.sync.dma_start(
                    xeT[:, kq * (KO // 4):(kq + 1) * (KO // 4), :],
                    xeT_d[:, kq * (KO // 4):(kq + 1) * (KO // 4)],
                )
            gw_sb = persist.tile([P, KO, E], F32, tag="gw")
            nc.sync.dma_start(gw_sb, gw_d[:])
            dsti_sb = persist.tile([P, NTC], I32, tag="dsti")
            nc.sync.dma_start(dsti_sb, dsti_d[:])
            rvi_sb = persist.tile([P, NCON * TOB], I32, tag="rvi")
            nc.sync.dma_start(rvi_sb, rvi_d[:])
            cbi_sb = persist.tile([P, NCON * TOB], I32, tag="cbi")
            nc.sync.dma_start(cbi_sb, cbi_d[:])
            ht_e = htp.tile([P, SLE, CAp + CBp], F16, tag="hte")
            for sect in range(2):
                Cs = CA if sect == 0 else CB
                coff = 0 if sect == 0 else CAp
                for i in range(SLE):
                    slab = wgup.tile(
                        [P, KO, 2 * P], F16, tag="slab", name=f"sl{sect}_{i}"
                    )
                    nc.sync.dma_start(slab, wgu_d[:, sect * SLE + i])
                    for (no, nw) in _chunks(Cs, NB1):
                        nsl = slice(coff + no, coff + no + nw)
                        pg = ps1.tile([P, NB1], F32, tag="pg", name=f"pg{sect}_{i}_{no}")
                        pu = ps1.tile([P, NB1], F32, tag="pu", name=f"pu{sect}_{i}_{no}")
                        for k in range(KO):
                            nc.tensor.matmul(
                                pg[:, :nw], slab[:, k, 0:P], xeT[:, k, nsl],
                                start=(k == 0), stop=(k == KO - 1),
                            )
                        for k in range(KO):
                            nc.tensor.matmul(
                                pu[:, :nw], slab[:, k, P:2 * P], xeT[:, k, nsl],
                                start=(k == 0), stop=(k == KO - 1),
                            )
                        sil = tmpp.tile([P, NB1], F16, tag="sil")
                        nc.scalar.activation(sil[:, :nw], pg[:, :nw], AF.Silu)
                        nc.vector.tensor_tensor(
                            ht_e[:, i, nsl], sil[:, :nw], pu[:, :nw], ALU.mult
                        )

            # ===== mm2 expert (half-I down partials) on dispatched tokens ==
            # One H-quarter per cc chunk: scatter each block's rows as they
            # finish and fire that quarter's AllToAll immediately, so all
            # collective traffic drains during mm2e (light DMA load) instead
            # of colliding with mm1b's weight streaming.
            for cc in range(HNC):
                subs = []
                for sect in range(2):
                    ss = []
                    for sub in range(2):
                        w = wdp.tile(
                            [P, SLE // 2, HN], F16, tag="wsl",
                            name=f"we{cc}_{sect}_{sub}",
                        )
                        nc.sync.dma_start(
                            w,
                            wde_d[:, sect, cc,
                                  sub * (SLE // 2):(sub + 1) * (SLE // 2)],
                        )
                        ss.append(w)
                    subs.append(ss)
                yesq = yesp.tile([P, NTC, HN], F16, tag="yes", name=f"yes{cc}")
                for bi in range(NTC):
                    sect = 0 if bi < NTCA else 1
                    col = bi * P if sect == 0 else CAp + (bi - NTCA) * P
                    py = ps2.tile([P, HN], F32, tag="py", name=f"pye{cc}_{bi}")
                    for k in range(SLE):
                        nc.tensor.matmul(
                            py, ht_e[:, k, col:col + P],
                            subs[sect][k // (SLE // 2)][:, k % (SLE // 2), :],
                            start=(k == 0), stop=(k == SLE - 1),
                        )
                    nc.scalar.activation(yesq[:, bi, :], py, AF.Copy)
                    nc.gpsimd.indirect_dma_start(
                        out=send_dram[cc][:],
                        out_offset=bass.IndirectOffsetOnAxis(
                            ap=dsti_sb[:, bi:bi + 1], axis=0
                        ),
                        in_=yesq[:, bi, :],
                        in_offset=None,
                        bounds_check=NCORE * SLOT - 1,
                        oob_is_err=False,
                    )
                nc.gpsimd.collective_compute(
                    "AllToAll",
                    ALU.bypass,
                    replica_groups=[list(range(NCORE))],
                    ins=[send_dram[cc][:]],
                    outs=[recv_dram[cc][:]],
                )

            # ============ router on own 256 tokens (strict fp32) ===========
            # logits^T: stationary = own x^T block [128h, 128tok], moving =
            # gw [128h, 8]; accumulate over k. One accumulation group at a
            # time per PSUM bank (start=True clears the whole bank's bits).
            zl_ps = psr.tile([P, TOB, E], F32, tag="zlps")
            for tb in range(TOB):
                xk = xk32p.tile([P, KO, P], F32, tag="xk")
                nc.sync.dma_start(xk, xrO_d[:, :, tb * P:(tb + 1) * P])
                for k in range(KO):
                    nc.tensor.matmul(
                        zl_ps[:, tb, :],
                        xk[:, k, :],
                        gw_sb[:, k, :],
                        start=(k == 0), stop=(k == KO - 1),
                    )
            zl = rsm.tile([P, TOB, E], F32, tag="zl")
            nc.vector.tensor_copy(zl, zl_ps)
            lmax = rsm.tile([P, TOB], F32, tag="lmax")
            nc.vector.reduce_max(lmax[:, :, None], zl, axis=AXX)
            nmax = rsm.tile([P, TOB], F32, tag="nmax")
            nc.vector.tensor_scalar_mul(nmax, lmax, -1.0)
            zex = rsm.tile([P, TOB, E], F32, tag="zex")
            for tb in range(TOB):
                nc.scalar.activation(
                    zex[:, tb, :], zl[:, tb, :], AF.Exp, bias=nmax[:, tb:tb + 1]
                )
            zlt = rsm.tile([P, TOB, E], F32, tag="zlt")
            nc.vector.tensor_scalar(zlt, zex, 1.0, None, op0=ALU.is_lt)
            zmk = rsm.tile([P, TOB, E], F32, tag="zmk")
            nc.vector.tensor_tensor(zmk, zex, zlt, ALU.mult)
            m2 = rsm.tile([P, TOB], F32, tag="m2")
            nc.vector.reduce_max(m2[:, :, None], zmk, axis=AXX)
            # per-expert top-2 mask and normalized weights: w_e =
            # zex_e * [zex_e >= m2] / (1 + m2)
            ge = rsm.tile([P, TOB, E], F32, tag="ge")
            nc.vector.tensor_tensor(
                ge, zex, m2[:, :, None].to_broadcast((P, TOB, E)), ALU.is_ge
            )
            s1 = rsm.tile([P, TOB], F32, tag="s1")
            nc.vector.tensor_scalar_add(s1, m2, 1.0)
            rcp = rsm.tile([P, TOB], F32, tag="rcp")
            nc.vector.reciprocal(rcp, s1)
            cw = rsm.tile([P, TOB, E], F32, tag="cw")
            nc.vector.tensor_tensor(cw, zex, ge, ALU.mult)
            cwn = rsm.tile([P, TOB, E], F32, tag="cwn")
            nc.vector.tensor_tensor(
                cwn, cw, rcp[:, :, None].to_broadcast((P, TOB, E)), ALU.mult
            )
            # store [TOWN*E, 1] with flat index (tb*128 + p)*8 + e
            nc.sync.dma_start(
                comb_dram[:].rearrange(
                    "(b p e) one -> p b (e one)", p=P, b=TOB, e=E
                ),
                cwn,
            )

            # prefetch combine-weight rows (router output, ready long ago)
            # before the gpsimd queue blocks on the collective
            cbs = []
            for sidx in range(NCON * TOB):
                cb = rgp.tile([P, 1], F32, tag=f"cb{sidx}", name=f"cb{sidx}")
                nc.gpsimd.indirect_dma_start(
                    out=cb[:],
                    out_offset=None,
                    in_=comb_dram[:],
                    in_offset=bass.IndirectOffsetOnAxis(
                        ap=cbi_sb[:, sidx:sidx + 1], axis=0
                    ),
                    bounds_check=TOWN * E - 1,
                    oob_is_err=False,
                )
                cbs.append(cb)
            # ============ mm1 base: own 256 tokens, full I =================
            xtO = xtp.tile([P, KO, TOWN], F16, tag="xt", name="xtO")
            nc.sync.dma_start(xtO, xtO_d[:])
            ht_b = htp.tile([P, IC_B, TOWN], F16, tag="hte", name="ht_b")
            for j in range(IC_B):
                slab = wgup.tile([P, KO, 2 * P], F16, tag="slab", name=f"slb{j}")
                slab_dma = nc.sync.dma_start(slab, wgu_d[:, 2 * SLE + j])
                if j == IC_B - 1:
                    last_mm1b_slab_dma = slab_dma
                pg = ps1.tile([P, TOWN], F32, tag="pg", name=f"bpg{j}")
                pu = ps1.tile([P, TOWN], F32, tag="pu", name=f"bpu{j}")
                # interleave gate/up so each LDWEIGHTS hides under the
                # previous matmul (N=256 leaves no slack otherwise)
                for k in range(KO):
                    nc.tensor.matmul(
                        pg, slab[:, k, 0:P], xtO[:, k, :],
                        start=(k == 0), stop=(k == KO - 1),
                    )
                    nc.tensor.matmul(
                        pu, slab[:, k, P:2 * P], xtO[:, k, :],
                        start=(k == 0), stop=(k == KO - 1),
                    )
                sil = tmpp.tile([P, TOWN], F16, tag="sil")
                nc.scalar.activation(sil, pg, AF.Silu)
                nc.vector.tensor_tensor(ht_b[:, j, :], sil, pu, ALU.mult)

            # ===== receive: gather 4 partial rows per token, accumulate ====
            # The weighted expert accumulation has no base dependency, so it
            # runs on the vector engine underneath the base down-projection.
            accs = [
                osbp.tile([P, H], F16, tag=f"osb{tb_}", name=f"osb{tb_}")
                for tb_ in range(TOB)
            ]
            for q in range(HNC):
                qsl = slice(q * HN, (q + 1) * HN)
                for tb in range(TOB):
                    for j in range(NCON):
                        sidx = j * TOB + tb
                        rg = rgp.tile([P, HN], F16, tag="rg")
                        rg_dma = nc.gpsimd.indirect_dma_start(
                            out=rg[:],
                            out_offset=None,
                            in_=recv_dram[q][:],
                            in_offset=bass.IndirectOffsetOnAxis(
                                ap=rvi_sb[:, sidx:sidx + 1], axis=0
                            ),
                            bounds_check=NCORE * SLOT - 1,
                            oob_is_err=False,
                        )
                        # Pin behind mm1b's last weight load: issued earlier,
                        # this gather's A2A-completion wait head-of-line
                        # blocks later DMAs sharing its completion lane.
                        bass._add_dep_helper(
                            rg_dma.ins, last_mm1b_slab_dma.ins, sync=True,
                            reason="defer recv gather",
                        )
                        if j == 0:
                            nc.vector.tensor_scalar_mul(
                                accs[tb][:, qsl], rg[:], cbs[sidx][:]
                            )
                        else:
                            nc.vector.scalar_tensor_tensor(
                                accs[tb][:, qsl], rg[:], cbs[sidx][:],
                                accs[tb][:, qsl], ALU.mult, ALU.add,
                            )

            # ============ mm2 base (down) on own tokens ====================
            base_lo = yesp.tile([P, TOB, HF], F16, tag="yes", name="base_lo")
            base_hi = xk32p.tile([P, TOB, HF], F16, tag="xk", name="base_hi")
            for cc in range(HNC):
                nsub = IC_B // (SLE // 2)
                bsubs = []
                for ss in range(nsub):
                    w = wdp.tile(
                        [P, SLE // 2, HN], F16, tag="wsl", name=f"wb{cc}_{ss}"
                    )
                    nc.sync.dma_start(
                        w, wdb_d[:, cc, ss * (SLE // 2):(ss + 1) * (SLE // 2)]
                    )
                    bsubs.append(w)
                for tb in range(TOB):
                    py = ps2.tile([P, HN], F32, tag="py", name=f"pyb{cc}_{tb}")
                    for j in range(IC_B):
                        nc.tensor.matmul(
                            py, ht_b[:, j, tb * P:(tb + 1) * P],
                            bsubs[j // (SLE // 2)][:, j % (SLE // 2), :],
                            start=(j == 0), stop=(j == IC_B - 1),
                        )
                    bdst = base_lo if cc < HNC // 2 else base_hi
                    bcc = cc % (HNC // 2)
                    nc.scalar.activation(
                        bdst[:, tb, bcc * HN:(bcc + 1) * HN], py, AF.Copy
                    )

            # ====== add base rows; low half first so it hides under the ====
            # ====== remaining base down-projection; write output shard =====
            for half in range(2):
                hsl = slice(half * HF, (half + 1) * HF)
                base_h = (base_lo, base_hi)[half]
                for tb in range(TOB):
                    nc.vector.tensor_tensor(
                        accs[tb][:, hsl], accs[tb][:, hsl], base_h[:, tb, :],
                        ALU.add,
                    )
            for tb in range(TOB):
                nc.sync.dma_start(out_d[tb], accs[tb])

    return nc


def _prep_inputs(x, gate_w, base_gate_up, base_down, expert_gate_up, expert_down):
    xf = np.ascontiguousarray(np.asarray(x, np.float32).reshape(NTOK, H))
    xT = np.ascontiguousarray(xf.reshape(NTOK, KO, P).transpose(2, 1, 0))
    xt16 = xT.astype(np.float16)
    gwf = np.asarray(gate_w, np.float32)
    gwp = np.ascontiguousarray(gwf.reshape(KO, P, E).transpose(1, 0, 2))

    # host-side dispatch: which tokens go to which expert (top-2 of logits)
    logits = xf @ gwf
    order = np.argsort(-logits, axis=1)
    top2 = order[:, :2]
    sel = [np.where((top2 == c).any(axis=1))[0].astype(np.int64) for c in range(E)]
    counts = np.array([len(s) for s in sel])

    # pair heavy experts with light ones so pair token sums are near-equal;
    # cores 2p / 2p+1 hold the low / high I-halves of pair p's two experts
    od = np.argsort(-counts, kind="stable")
    pairs = [(int(od[i]), int(od[E - 1 - i])) for i in range(NPAIR)]
    e2pr = {}
    for pi, (a, b) in enumerate(pairs):
        e2pr[a] = (pi, 0)
        e2pr[b] = (pi, 1)

    CA = int(max(counts[a] for a, b in pairs))
    CB = int(max(counts[b] for a, b in pairs))
    NTCA = (CA + P - 1) // P
    NTCB = (CB + P - 1) // P
    CAp, CBp = NTCA * P, NTCB * P
    NTC = NTCA + NTCB

    # per-pair concatenated token columns: [a tokens|pad] + [b tokens|pad];
    # send position = order of appearance within the (pair -> home) group
    pair_flat = []
    pair_pos = []
    pair_colof = []
    max_grp = 0
    for pi, (a, b) in enumerate(pairs):
        La, Lb = sel[a], sel[b]
        flat = np.full(CAp + CBp, -1, np.int64)
        flat[: len(La)] = La
        flat[CAp:CAp + len(Lb)] = Lb
        real = np.nonzero(flat >= 0)[0]
        pos = np.full(CAp + CBp, OOB_IDX, np.int64)
        cnt = np.zeros(NCORE, np.int64)
        colof = {}
        for ci in real:
            hm = flat[ci] // TOWN
            pos[ci] = cnt[hm]
            cnt[hm] += 1
            colof[(int(flat[ci]), 0 if ci < CAp else 1)] = ci
        max_grp = max(max_grp, int(cnt.max()))
        pair_flat.append(flat)
        pair_pos.append(pos)
        pair_colof.append(colof)
    SLOT = (max_grp + 3) // 4 * 4

    # per-pair send index: column (block bi, partition p) -> home*SLOT + pos
    dsti_p = []
    for pi in range(NPAIR):
        flat, pos = pair_flat[pi], pair_pos[pi]
        dst = np.where(
            flat >= 0, (flat // TOWN) * SLOT + pos, OOB_IDX
        ).astype(np.int64)
        dsti_p.append(
            np.ascontiguousarray(dst.reshape(NTC, P).T.astype(np.int32))
        )

    # per-core receive index: own token t, contribution j in 0..3 =
    # (expert rank j//2 sorted, I-half j%2): recv row = sender*SLOT + pos
    rvi = np.zeros((NCORE, P, NCON * TOB), np.int32)
    cbi = np.zeros((NCORE, P, NCON * TOB), np.int32)
    for hme in range(NCORE):
        for tl in range(TOWN):
            t = hme * TOWN + tl
            tb, p = divmod(tl, P)
            es = np.sort(top2[t])
            for j2, e in enumerate(es):
                pi, role = e2pr[int(e)]
                ci = pair_colof[pi][(t, role)]
                ps_ = int(pair_pos[pi][ci])
                for half in range(2):
                    j = j2 * 2 + half
                    sender = 2 * pi + half
                    rvi[hme, p, j * TOB + tb] = sender * SLOT + ps_
                    cbi[hme, p, j * TOB + tb] = tl * E + e

    bgu = np.asarray(base_gate_up, np.float32)
    gb_ = bgu[:, :I_EXP].reshape(H, IC_B, P)
    ub_ = bgu[:, I_EXP:].reshape(H, IC_B, P)
    pb_ = np.concatenate([gb_, ub_], axis=2)  # [H, IC_B, 2P]
    bd = np.asarray(base_down, np.float32)
    wdb_p = np.ascontiguousarray(
        bd.reshape(IC_B, P, HNC, HN).transpose(1, 2, 0, 3)
    ).astype(np.float16)

    in_maps = []
    for c in range(NCORE):
        pi, half = divmod(c, 2)
        a, b = pairs[pi]

        def half_slabs(We):
            We = np.asarray(We, np.float32)
            g = We[:, half * HI:(half + 1) * HI].reshape(H, SLE, P)
            u = We[:, I_EXP + half * HI:I_EXP + (half + 1) * HI].reshape(
                H, SLE, P
            )
            return np.concatenate([g, u], axis=2)  # [H, SLE, 2P]

        allp = np.concatenate(
            [half_slabs(expert_gate_up[a]), half_slabs(expert_gate_up[b]), pb_],
            axis=1,
        )  # [H, ICT, 2P]
        wgu_p = np.ascontiguousarray(
            allp.reshape(KO, P, ICT, 2 * P).transpose(1, 2, 0, 3)
        ).astype(np.float16)

        def half_down(ed_):
            d = np.asarray(ed_, np.float32)[half * HI:(half + 1) * HI]
            return d.reshape(SLE, P, HNC, HN).transpose(1, 2, 0, 3)

        wde_p = np.ascontiguousarray(
            np.stack(
                [half_down(expert_down[a]), half_down(expert_down[b])], axis=1
            )
        ).astype(np.float16)  # [P, 2, HNC, SLE, HN]

        flat = pair_flat[pi]
        real = flat >= 0
        xe = np.zeros((P, KO, CAp + CBp), np.float16)
        xe[:, :, real] = xt16[:, :, flat[real]]
        own = slice(c * TOWN, (c + 1) * TOWN)
        in_maps.append(
            dict(
                xeT=np.ascontiguousarray(xe),
                xtO=np.ascontiguousarray(xt16[:, :, own]),
                xrO=np.ascontiguousarray(xT[:, :, own]),
                wgu=wgu_p, wde=wde_p, wdb=wdb_p, gw=gwp,
                dsti=dsti_p[pi], rvi=rvi[c], cbi=cbi[c],
            )
        )
    return in_maps, CA, CB, SLOT


LAST_RESULTS = None


def kernel(x, gate_w, base_gate_up, base_down, expert_gate_up, expert_down):
    global LAST_RESULTS
    in_maps, CA, CB, SLOT = _prep_inputs(
        x, gate_w, base_gate_up, base_down, expert_gate_up, expert_down
    )
    nc = _build(CA, CB, SLOT)
    if not nc.is_finalized():
        nc.finalize()
    res = run_bass_kernel_spmd(nc, in_maps, core_ids=list(range(NCORE)))
    LAST_RESULTS = res
    y = np.empty((NTOK, H), np.float32)
    for c in range(NCORE):
        o = res.results[c]["out"]  # [TOB, P, H] f16
        y[c * TOWN:(c + 1) * TOWN] = o.reshape(TOWN, H).astype(np.float32)
    return y.reshape(1, NTOK, H)


if __name__ == "__main__":
    nc = _build(545, 510, 160)
    print("build ok; instructions:",
          sum(len(b.instructions) for b in nc.main_func.blocks))


# revision 12
# speedup vs baseline: 1.0923x; 1.0146x over previous
"""LlamaMoE (H=2048, I=4096, E=8 experts, top-2, N=2048 tokens) on 8 trn2 cores.

Strategy: PAIR-SPLIT expert parallelism + token-parallel base MLP, combined
with a single split AllToAll.

The tensor engine is power-throttled to ~1.9 GHz sustained and the baseline
schedule had zero tensor idle, so the only win is fewer matmul cycles.
Expert token counts are imbalanced (484..545 vs 512 avg); with one expert
per core every core pays for the worst count. Instead, experts are PAIRED
large-with-small (sums 1019..1029, +-0.5%): the two cores of a pair each
hold HALF of the intermediate dim (I/2) of BOTH experts and process the
pair's full token list. Each (token, expert) down-projection row is then a
half-I partial computed on two cores; both partials ride the AllToAll and
the home core sums 4 contributions (2 experts x 2 halves) instead of 2.
Per-core matmul cycles drop ~6% and are balanced regardless of routing
skew, with identical weight DMA volume.

Host supplies the dispatch permutation (pre-gathered transposed activations
plus send/receive index maps padded with OOB sentinels); all model math --
router logits, top-2 combine weights, expert MLPs, base MLP, combine --
runs on device. The base MLP stays row-sharded: core c computes the full
base MLP for its own 256 token rows (no cross-core reduction).

Expert partial rows are scattered into an AllToAll send buffer grouped by
destination (token-home) core; one fp16 AllToAll per H-quarter fires as
soon as that column chunk of the down-projection completes, so all four
collectives drain during mm2e itself (light DMA phase) instead of starving
mm1b's weight stream. The home core computes the router (fp32) on its own tokens for the
top-2 combine weights, accumulates the 4 weighted contribution rows per
token (no base dependency), adds the base rows when they land, and writes
its 256-row output shard.
"""

import numpy as np

import concourse.bacc as bacc
import concourse.bass as bass
import concourse.mybir as mybir
import concourse.tile as tile
from concourse.bass_utils import run_bass_kernel_spmd

P = 128
H = 2048
I_EXP = 4096
HI = I_EXP // 2             # half intermediate dim per core
E = 8
NCORE = 8
NPAIR = NCORE // 2
NTOK = 2048
TOWN = NTOK // NCORE        # 256 own token rows per core
TOB = TOWN // P             # 2 own token blocks
KO = H // P                 # 16 contraction tiles for mm1
SLE = HI // P               # 16 half-I slabs per expert section
IC_B = I_EXP // P           # 32 base chunks (full I, row-sharded base)
ICT = 2 * SLE + IC_B        # 64 gate/up slabs (a-half, b-half, base)
NB1 = 512                   # mm1 expert moving free dim (tokens)
HN = 512                    # mm2 moving free dim (H cols) = A2A quarter width
HNC = H // HN               # 4
HF = H // 2                 # column half (base mm2 lo/hi split)
NCON = 4                    # combine contributions per token (2 exp x 2 half)

F32 = mybir.dt.float32
F16 = mybir.dt.float16
I32 = mybir.dt.int32
AF = mybir.ActivationFunctionType
ALU = mybir.AluOpType
AXX = mybir.AxisListType.X

OOB_IDX = 1 << 20


def _chunks(total, step):
    out = []
    o = 0
    while o < total:
        out.append((o, min(step, total - o)))
        o += step
    return out


def _build(CA, CB, SLOT):
    NTCA = (CA + P - 1) // P
    NTCB = (CB + P - 1) // P
    NTC = NTCA + NTCB
    CAp, CBp = NTCA * P, NTCB * P
    nc = bacc.Bacc(None)
    xeT_d = nc.dram_tensor("xeT", [P, KO, CAp + CBp], F16, kind="ExternalInput")
    xtO_d = nc.dram_tensor("xtO", [P, KO, TOWN], F16, kind="ExternalInput")
    xrO_d = nc.dram_tensor("xrO", [P, KO, TOWN], F32, kind="ExternalInput")
    wgu_d = nc.dram_tensor("wgu", [P, ICT, KO, 2 * P], F16, kind="ExternalInput")
    wde_d = nc.dram_tensor("wde", [P, 2, HNC, SLE, HN], F16, kind="ExternalInput")
    wdb_d = nc.dram_tensor("wdb", [P, HNC, IC_B, HN], F16, kind="ExternalInput")
    gw_d = nc.dram_tensor("gw", [P, KO, E], F32, kind="ExternalInput")
    dsti_d = nc.dram_tensor("dsti", [P, NTC], I32, kind="ExternalInput")
    rvi_d = nc.dram_tensor("rvi", [P, NCON * TOB], I32, kind="ExternalInput")
    cbi_d = nc.dram_tensor("cbi", [P, NCON * TOB], I32, kind="ExternalInput")
    out_d = nc.dram_tensor("out", [TOB, P, H], F16, kind="ExternalOutput")

    from contextlib import ExitStack
    with tile.TileContext(nc) as tc:
        with ExitStack() as _stk:
            def _pool(**kw):
                return _stk.enter_context(tc.tile_pool(**kw))
            persist = _pool(name="persist", bufs=1)
            xtp = _pool(name="xt", bufs=1)
            htp = _pool(name="ht", bufs=1)
            wgup = _pool(name="wgup", bufs=6)
            wdp = _pool(name="wdp", bufs=4)
            xk32p = _pool(name="xk32", bufs=1)
            tmpp = _pool(name="tmp", bufs=2)
            yesp = _pool(name="yesp", bufs=2)
            rgp = _pool(name="rgp", bufs=3)
            osbp = _pool(name="osb", bufs=1)
            rsm = _pool(name="rsm", bufs=1)
            ps1 = _pool(name="ps1", bufs=2, space="PSUM")
            ps2 = _pool(name="ps2", bufs=2, space="PSUM")
            psr = _pool(name="psr", bufs=1, space="PSUM")
            dram = _pool(name="dram", bufs=1, space="DRAM")
            send_dram = [
                dram.tile([NCORE * SLOT, HN], F16, tag=f"send{q_}", name=f"send{q_}")
                for q_ in range(HNC)
            ]
            recv_dram = [
                dram.tile([NCORE * SLOT, HN], F16, tag=f"recv{q_}", name=f"recv{q_}")
                for q_ in range(HNC)
            ]
            comb_dram = dram.tile([TOWN * E, 1], F32, tag="combd")

            # ===== mm1 expert: gate/up + silu*up on both half-experts ======
            # xeT columns: [pair-expert-a tokens | pad | expert-b tokens |
            # pad]; pads are zero so h comes out 0 and the rows map to OOB
            # send slots downstream. xeT arrives in k-chunks so the first
            # matmuls start as soon as chunk 0 lands.
            xeT = xtp.tile([P, KO, CAp + CBp], F16, tag="xt", name="xeT")
            for kq in range(4):
                nc.sync.dma_start(
                    xeT[:, kq * (KO // 4):(kq + 1) * (KO // 4), :],
                    xeT_d[:, kq * (KO // 4):(kq + 1) * (KO // 4)],
                )
            gw_sb = persist.tile([P, KO, E], F32, tag="gw")
            nc.sync.dma_start(gw_sb, gw_d[:])
            dsti_sb = persist.tile([P, NTC], I32, tag="dsti")
            nc.sync.dma_start(dsti_sb, dsti_d[:])
            rvi_sb = persist.tile([P, NCON * TOB], I32, tag="rvi")
            nc.sync.dma_start(rvi_sb, rvi_d[:])
            cbi_sb = persist.tile([P, NCON * TOB], I32, tag="cbi")
            nc.sync.dma_start(cbi_sb, cbi_d[:])
            ht_e = htp.tile([P, SLE, CAp + CBp], F16, tag="hte")
            for sect in range(2):
                Cs = CA if sect == 0 else CB
                coff = 0 if sect == 0 else CAp
                for i in range(SLE):
                    slab = wgup.tile(
                        [P, KO, 2 * P], F16, tag="slab", name=f"sl{sect}_{i}"
                    )
                    nc.sync.dma_start(slab, wgu_d[:, sect * SLE + i])
                    for (no, nw) in _chunks(Cs, NB1):
                        nsl = slice(coff + no, coff + no + nw)
                        pg = ps1.tile([P, NB1], F32, tag="pg", name=f"pg{sect}_{i}_{no}")
                        pu = ps1.tile([P, NB1], F32, tag="pu", name=f"pu{sect}_{i}_{no}")
                        for k in range(KO):
                            nc.tensor.matmul(
                                pg[:, :nw], slab[:, k, 0:P], xeT[:, k, nsl],
                                start=(k == 0), stop=(k == KO - 1),
                            )
                        for k in range(KO):
                            nc.tensor.matmul(
                                pu[:, :nw], slab[:, k, P:2 * P], xeT[:, k, nsl],
                                start=(k == 0), stop=(k == KO - 1),
                            )
                        sil = tmpp.tile([P, NB1], F16, tag="sil")
                        nc.scalar.activation(sil[:, :nw], pg[:, :nw], AF.Silu)
                        nc.vector.tensor_tensor(
                            ht_e[:, i, nsl], sil[:, :nw], pu[:, :nw], ALU.mult
                        )

            # ===== mm2 expert (half-I down partials) on dispatched tokens ==
            # One H-quarter per cc chunk: scatter each block's rows as they
            # finish and fire that quarter's AllToAll immediately, so all
            # collective traffic drains during mm2e (light DMA load) instead
            # of colliding with mm1b's weight streaming.
            for cc in range(HNC):
                subs = []
                for sect in range(2):
                    ss = []
                    for sub in range(2):
                        w = wdp.tile(
                            [P, SLE // 2, HN], F16, tag="wsl",
                            name=f"we{cc}_{sect}_{sub}",
                        )
                        nc.sync.dma_start(
                            w,
                            wde_d[:, sect, cc,
                                  sub * (SLE // 2):(sub + 1) * (SLE // 2)],
                        )
                        ss.append(w)
                    subs.append(ss)
                yesq = yesp.tile([P, NTC, HN], F16, tag="yes", name=f"yes{cc}")
                for bi in range(NTC):
                    sect = 0 if bi < NTCA else 1
                    col = bi * P if sect == 0 else CAp + (bi - NTCA) * P
                    py = ps2.tile([P, HN], F32, tag="py", name=f"pye{cc}_{bi}")
                    for k in range(SLE):
                        nc.tensor.matmul(
                            py, ht_e[:, k, col:col + P],
                            subs[sect][k // (SLE // 2)][:, k % (SLE // 2), :],
                            start=(k == 0), stop=(k == SLE - 1),
                        )
                    nc.scalar.activation(yesq[:, bi, :], py, AF.Copy)
                    nc.gpsimd.indirect_dma_start(
                        out=send_dram[cc][:],
                        out_offset=bass.IndirectOffsetOnAxis(
                            ap=dsti_sb[:, bi:bi + 1], axis=0
                        ),
                        in_=yesq[:, bi, :],
                        in_offset=None,
                        bounds_check=NCORE * SLOT - 1,
                        oob_is_err=False,
                    )
                nc.gpsimd.collective_compute(
                    "AllToAll",
                    ALU.bypass,
                    replica_groups=[list(range(NCORE))],
                    ins=[send_dram[cc][:]],
                    outs=[recv_dram[cc][:]],
                )

            # ============ router on own 256 tokens (strict fp32) ===========
            # logits^T: stationary = own x^T block [128h, 128tok], moving =
            # gw [128h, 8]; accumulate over k. One accumulation group at a
            # time per PSUM bank (start=True clears the whole bank's bits).
            zl_ps = psr.tile([P, TOB, E], F32, tag="zlps")
            for tb in range(TOB):
                xk = xk32p.tile([P, KO, P], F32, tag="xk")
                nc.sync.dma_start(xk, xrO_d[:, :, tb * P:(tb + 1) * P])
                for k in range(KO):
                    nc.tensor.matmul(
                        zl_ps[:, tb, :],
                        xk[:, k, :],
                        gw_sb[:, k, :],
                        start=(k == 0), stop=(k == KO - 1),
                    )
            zl = rsm.tile([P, TOB, E], F32, tag="zl")
            nc.vector.tensor_copy(zl, zl_ps)
            lmax = rsm.tile([P, TOB], F32, tag="lmax")
            nc.vector.reduce_max(lmax[:, :, None], zl, axis=AXX)
            nmax = rsm.tile([P, TOB], F32, tag="nmax")
            nc.vector.tensor_scalar_mul(nmax, lmax, -1.0)
            zex = rsm.tile([P, TOB, E], F32, tag="zex")
            for tb in range(TOB):
                nc.scalar.activation(
                    zex[:, tb, :], zl[:, tb, :], AF.Exp, bias=nmax[:, tb:tb + 1]
                )
            zlt = rsm.tile([P, TOB, E], F32, tag="zlt")
            nc.vector.tensor_scalar(zlt, zex, 1.0, None, op0=ALU.is_lt)
            zmk = rsm.tile([P, TOB, E], F32, tag="zmk")
            nc.vector.tensor_tensor(zmk, zex, zlt, ALU.mult)
            m2 = rsm.tile([P, TOB], F32, tag="m2")
            nc.vector.reduce_max(m2[:, :, None], zmk, axis=AXX)
            # per-expert top-2 mask and normalized weights: w_e =
            # zex_e * [zex_e >= m2] / (1 + m2)
            ge = rsm.tile([P, TOB, E], F32, tag="ge")
            nc.vector.tensor_tensor(
                ge, zex, m2[:, :, None].to_broadcast((P, TOB, E)), ALU.is_ge
            )
            s1 = rsm.tile([P, TOB], F32, tag="s1")
            nc.vector.tensor_scalar_add(s1, m2, 1.0)
            rcp = rsm.tile([P, TOB], F32, tag="rcp")
            nc.vector.reciprocal(rcp, s1)
            cw = rsm.tile([P, TOB, E], F32, tag="cw")
            nc.vector.tensor_tensor(cw, zex, ge, ALU.mult)
            cwn = rsm.tile([P, TOB, E], F32, tag="cwn")
            nc.vector.tensor_tensor(
                cwn, cw, rcp[:, :, None].to_broadcast((P, TOB, E)), ALU.mult
            )
            # store [TOWN*E, 1] with flat index (tb*128 + p)*8 + e
            nc.sync.dma_start(
                comb_dram[:].rearrange(
                    "(b p e) one -> p b (e one)", p=P, b=TOB, e=E
                ),
                cwn,
            )

            # prefetch combine-weight rows (router output, ready long ago)
            # before the gpsimd queue blocks on the collective
            cbs = []
            for sidx in range(NCON * TOB):
                cb = rgp.tile([P, 1], F32, tag=f"cb{sidx}", name=f"cb{sidx}")
                nc.gpsimd.indirect_dma_start(
                    out=cb[:],
                    out_offset=None,
                    in_=comb_dram[:],
                    in_offset=bass.IndirectOffsetOnAxis(
                        ap=cbi_sb[:, sidx:sidx + 1], axis=0
                    ),
                    bounds_check=TOWN * E - 1,
                    oob_is_err=False,
                )
                cbs.append(cb)
            # ============ mm1 base: own 256 tokens, full I =================
            xtO = xtp.tile([P, KO, TOWN], F16, tag="xt", name="xtO")
            nc.sync.dma_start(xtO, xtO_d[:])
            ht_b = htp.tile([P, IC_B, TOWN], F16, tag="hte", name="ht_b")
            for j in range(IC_B):
                slab = wgup.tile([P, KO, 2 * P], F16, tag="slab", name=f"slb{j}")
                slab_dma = nc.sync.dma_start(slab, wgu_d[:, 2 * SLE + j])
                if j == IC_B - 1:
                    last_mm1b_slab_dma = slab_dma
                pg = ps1.tile([P, TOWN], F32, tag="pg", name=f"bpg{j}")
                pu = ps1.tile([P, TOWN], F32, tag="pu", name=f"bpu{j}")
                # interleave gate/up so each LDWEIGHTS hides under the
                # previous matmul (N=256 leaves no slack otherwise)
                for k in range(KO):
                    nc.tensor.matmul(
                        pg, slab[:, k, 0:P], xtO[:, k, :],
                        start=(k == 0), stop=(k == KO - 1),
                    )
                    nc.tensor.matmul(
                        pu, slab[:, k, P:2 * P], xtO[:, k, :],
                        start=(k == 0), stop=(k == KO - 1),
                    )
                sil = tmpp.tile([P, TOWN], F16, tag="sil")
                nc.scalar.activation(sil, pg, AF.Silu)
                nc.vector.tensor_tensor(ht_b[:, j, :], sil, pu, ALU.mult)
                if j == 22:
                    # prefetch mm2b's first column chunk of base down weights
                    # here: emitted later they would queue behind the rest of
                    # the slab DMAs and stall mm2b's first matmuls
                    wdb0 = []
                    for ss in range(IC_B // (SLE // 2)):
                        w = wdp.tile(
                            [P, SLE // 2, HN], F16, tag="wsl", name=f"wb0_{ss}"
                        )
                        nc.sync.dma_start(
                            w,
                            wdb_d[:, 0, ss * (SLE // 2):(ss + 1) * (SLE // 2)],
                        )
                        wdb0.append(w)

            # ===== receive: gather 4 partial rows per token, accumulate ====
            # The weighted expert accumulation has no base dependency, so it
            # runs on the vector engine underneath the base down-projection.
            accs = [
                osbp.tile([P, H], F16, tag=f"osb{tb_}", name=f"osb{tb_}")
                for tb_ in range(TOB)
            ]
            for q in range(HNC):
                qsl = slice(q * HN, (q + 1) * HN)
                for tb in range(TOB):
                    for j in range(NCON):
                        sidx = j * TOB + tb
                        rg = rgp.tile([P, HN], F16, tag="rg")
                        rg_dma = nc.gpsimd.indirect_dma_start(
                            out=rg[:],
                            out_offset=None,
                            in_=recv_dram[q][:],
                            in_offset=bass.IndirectOffsetOnAxis(
                                ap=rvi_sb[:, sidx:sidx + 1], axis=0
                            ),
                            bounds_check=NCORE * SLOT - 1,
                            oob_is_err=False,
                        )
                        # Pin behind mm1b's last weight load: issued earlier,
                        # this gather's A2A-completion wait head-of-line
                        # blocks later DMAs sharing its completion lane.
                        bass._add_dep_helper(
                            rg_dma.ins, last_mm1b_slab_dma.ins, sync=True,
                            reason="defer recv gather",
                        )
                        if j == 0:
                            nc.vector.tensor_scalar_mul(
                                accs[tb][:, qsl], rg[:], cbs[sidx][:]
                            )
                        else:
                            nc.vector.scalar_tensor_tensor(
                                accs[tb][:, qsl], rg[:], cbs[sidx][:],
                                accs[tb][:, qsl], ALU.mult, ALU.add,
                            )

            # ============ mm2 base (down) on own tokens ====================
            base_lo = yesp.tile([P, TOB, HF], F16, tag="yes", name="base_lo")
            base_hi = xk32p.tile([P, TOB, HF], F16, tag="xk", name="base_hi")
            for cc in range(HNC):
                nsub = IC_B // (SLE // 2)
                if cc == 0:
                    bsubs = wdb0
                else:
                    bsubs = []
                    for ss in range(nsub):
                        w = wdp.tile(
                            [P, SLE // 2, HN], F16, tag="wsl", name=f"wb{cc}_{ss}"
                        )
                        nc.sync.dma_start(
                            w, wdb_d[:, cc, ss * (SLE // 2):(ss + 1) * (SLE // 2)]
                        )
                        bsubs.append(w)
                for tb in range(TOB):
                    py = ps2.tile([P, HN], F32, tag="py", name=f"pyb{cc}_{tb}")
                    for j in range(IC_B):
                        nc.tensor.matmul(
                            py, ht_b[:, j, tb * P:(tb + 1) * P],
                            bsubs[j // (SLE // 2)][:, j % (SLE // 2), :],
                            start=(j == 0), stop=(j == IC_B - 1),
                        )
                    bdst = base_lo if cc < HNC // 2 else base_hi
                    bcc = cc % (HNC // 2)
                    nc.scalar.activation(
                        bdst[:, tb, bcc * HN:(bcc + 1) * HN], py, AF.Copy
                    )

            # ====== add base rows; low half first so it hides under the ====
            # ====== remaining base down-projection; write output shard =====
            for half in range(2):
                hsl = slice(half * HF, (half + 1) * HF)
                base_h = (base_lo, base_hi)[half]
                for tb in range(TOB):
                    nc.vector.tensor_tensor(
                        accs[tb][:, hsl], accs[tb][:, hsl], base_h[:, tb, :],
                        ALU.add,
                    )
            for tb in range(TOB):
                nc.sync.dma_start(out_d[tb], accs[tb])

    return nc


def _prep_inputs(x, gate_w, base_gate_up, base_down, expert_gate_up, expert_down):
    xf = np.ascontiguousarray(np.asarray(x, np.float32).reshape(NTOK, H))
    xT = np.ascontiguousarray(xf.reshape(NTOK, KO, P).transpose(2, 1, 0))
    xt16 = xT.astype(np.float16)
    gwf = np.asarray(gate_w, np.float32)
    gwp = np.ascontiguousarray(gwf.reshape(KO, P, E).transpose(1, 0, 2))

    # host-side dispatch: which tokens go to which expert (top-2 of logits)
    logits = xf @ gwf
    order = np.argsort(-logits, axis=1)
    top2 = order[:, :2]
    sel = [np.where((top2 == c).any(axis=1))[0].astype(np.int64) for c in range(E)]
    counts = np.array([len(s) for s in sel])

    # pair heavy experts with light ones so pair token sums are near-equal;
    # cores 2p / 2p+1 hold the low / high I-halves of pair p's two experts
    od = np.argsort(-counts, kind="stable")
    pairs = [(int(od[i]), int(od[E - 1 - i])) for i in range(NPAIR)]
    e2pr = {}
    for pi, (a, b) in enumerate(pairs):
        e2pr[a] = (pi, 0)
        e2pr[b] = (pi, 1)

    CA = int(max(counts[a] for a, b in pairs))
    CB = int(max(counts[b] for a, b in pairs))
    NTCA = (CA + P - 1) // P
    NTCB = (CB + P - 1) // P
    CAp, CBp = NTCA * P, NTCB * P
    NTC = NTCA + NTCB

    # per-pair concatenated token columns: [a tokens|pad] + [b tokens|pad];
    # send position = order of appearance within the (pair -> home) group
    pair_flat = []
    pair_pos = []
    pair_colof = []
    max_grp = 0
    for pi, (a, b) in enumerate(pairs):
        La, Lb = sel[a], sel[b]
        flat = np.full(CAp + CBp, -1, np.int64)
        flat[: len(La)] = La
        flat[CAp:CAp + len(Lb)] = Lb
        real = np.nonzero(flat >= 0)[0]
        pos = np.full(CAp + CBp, OOB_IDX, np.int64)
        cnt = np.zeros(NCORE, np.int64)
        colof = {}
        for ci in real:
            hm = flat[ci] // TOWN
            pos[ci] = cnt[hm]
            cnt[hm] += 1
            colof[(int(flat[ci]), 0 if ci < CAp else 1)] = ci
        max_grp = max(max_grp, int(cnt.max()))
        pair_flat.append(flat)
        pair_pos.append(pos)
        pair_colof.append(colof)
    SLOT = (max_grp + 3) // 4 * 4

    # per-pair send index: column (block bi, partition p) -> home*SLOT + pos
    dsti_p = []
    for pi in range(NPAIR):
        flat, pos = pair_flat[pi], pair_pos[pi]
        dst = np.where(
            flat >= 0, (flat // TOWN) * SLOT + pos, OOB_IDX
        ).astype(np.int64)
        dsti_p.append(
            np.ascontiguousarray(dst.reshape(NTC, P).T.astype(np.int32))
        )

    # per-core receive index: own token t, contribution j in 0..3 =
    # (expert rank j//2 sorted, I-half j%2): recv row = sender*SLOT + pos
    rvi = np.zeros((NCORE, P, NCON * TOB), np.int32)
    cbi = np.zeros((NCORE, P, NCON * TOB), np.int32)
    for hme in range(NCORE):
        for tl in range(TOWN):
            t = hme * TOWN + tl
            tb, p = divmod(tl, P)
            es = np.sort(top2[t])
            for j2, e in enumerate(es):
                pi, role = e2pr[int(e)]
                ci = pair_colof[pi][(t, role)]
                ps_ = int(pair_pos[pi][ci])
                for half in range(2):
                    j = j2 * 2 + half
                    sender = 2 * pi + half
                    rvi[hme, p, j * TOB + tb] = sender * SLOT + ps_
                    cbi[hme, p, j * TOB + tb] = tl * E + e

    bgu = np.asarray(base_gate_up, np.float32)
    gb_ = bgu[:, :I_EXP].reshape(H, IC_B, P)
    ub_ = bgu[:, I_EXP:].reshape(H, IC_B, P)
    pb_ = np.concatenate([gb_, ub_], axis=2)  # [H, IC_B, 2P]
    bd = np.asarray(base_down, np.float32)
    wdb_p = np.ascontiguousarray(
        bd.reshape(IC_B, P, HNC, HN).transpose(1, 2, 0, 3)
    ).astype(np.float16)

    in_maps = []
    for c in range(NCORE):
        pi, half = divmod(c, 2)
        a, b = pairs[pi]

        def half_slabs(We):
            We = np.asarray(We, np.float32)
            g = We[:, half * HI:(half + 1) * HI].reshape(H, SLE, P)
            u = We[:, I_EXP + half * HI:I_EXP + (half + 1) * HI].reshape(
                H, SLE, P
            )
            return np.concatenate([g, u], axis=2)  # [H, SLE, 2P]

        allp = np.concatenate(
            [half_slabs(expert_gate_up[a]), half_slabs(expert_gate_up[b]), pb_],
            axis=1,
        )  # [H, ICT, 2P]
        wgu_p = np.ascontiguousarray(
            allp.reshape(KO, P, ICT, 2 * P).transpose(1, 2, 0, 3)
        ).astype(np.float16)

        def half_down(ed_):
            d = np.asarray(ed_, np.float32)[half * HI:(half + 1) * HI]
            return d.reshape(SLE, P, HNC, HN).transpose(1, 2, 0, 3)

        wde_p = np.ascontiguousarray(
            np.stack(
                [half_down(expert_down[a]), half_down(expert_down[b])], axis=1
            )
        ).astype(np.float16)  # [P, 2, HNC, SLE, HN]

        flat = pair_flat[pi]
        real = flat >= 0
        xe = np.zeros((P, KO, CAp + CBp), np.float16)
        xe[:, :, real] = xt16[:, :, flat[real]]
        own = slice(c * TOWN, (c + 1) * TOWN)
        in_maps.append(
            dict(
                xeT=np.ascontiguousarray(xe),
                xtO=np.ascontiguousarray(xt16[:, :, own]),
                xrO=np.ascontiguousarray(xT[:, :, own]),
                wgu=wgu_p, wde=wde_p, wdb=wdb_p, gw=gwp,
                dsti=dsti_p[pi], rvi=rvi[c], cbi=cbi[c],
            )
        )
    return in_maps, CA, CB, SLOT


LAST_RESULTS = None


def kernel(x, gate_w, base_gate_up, base_down, expert_gate_up, expert_down):
    global LAST_RESULTS
    in_maps, CA, CB, SLOT = _prep_inputs(
        x, gate_w, base_gate_up, base_down, expert_gate_up, expert_down
    )
    nc = _build(CA, CB, SLOT)
    if not nc.is_finalized():
        nc.finalize()
    res = run_bass_kernel_spmd(nc, in_maps, core_ids=list(range(NCORE)))
    LAST_RESULTS = res
    y = np.empty((NTOK, H), np.float32)
    for c in range(NCORE):
        o = res.results[c]["out"]  # [TOB, P, H] f16
        y[c * TOWN:(c + 1) * TOWN] = o.reshape(TOWN, H).astype(np.float32)
    return y.reshape(1, NTOK, H)


if __name__ == "__main__":
    nc = _build(545, 510, 160)
    print("build ok; instructions:",
          sum(len(b.instructions) for b in nc.main_func.blocks))


# revision 16
# speedup vs baseline: 1.0924x; 1.0001x over previous
"""LlamaMoE (H=2048, I=4096, E=8 experts, top-2, N=2048 tokens) on 8 trn2 cores.

Strategy: PAIR-SPLIT expert parallelism + token-parallel base MLP, combined
with a single split AllToAll.

The tensor engine is power-throttled to ~1.9 GHz sustained and the baseline
schedule had zero tensor idle, so the only win is fewer matmul cycles.
Expert token counts are imbalanced (484..545 vs 512 avg); with one expert
per core every core pays for the worst count. Instead, experts are PAIRED
large-with-small (sums 1019..1029, +-0.5%): the two cores of a pair each
hold HALF of the intermediate dim (I/2) of BOTH experts and process the
pair's full token list. Each (token, expert) down-projection row is then a
half-I partial computed on two cores; both partials ride the AllToAll and
the home core sums 4 contributions (2 experts x 2 halves) instead of 2.
Per-core matmul cycles drop ~6% and are balanced regardless of routing
skew, with identical weight DMA volume.

Host supplies the dispatch permutation (pre-gathered transposed activations
plus send/receive index maps padded with OOB sentinels); all model math --
router logits, top-2 combine weights, expert MLPs, base MLP, combine --
runs on device. The base MLP stays row-sharded: core c computes the full
base MLP for its own 256 token rows (no cross-core reduction).

Expert partial rows are scattered into an AllToAll send buffer grouped by
destination (token-home) core; one fp16 AllToAll per H-quarter fires as
soon as that column chunk of the down-projection completes, so all four
collectives drain during mm2e itself (light DMA phase) instead of starving
mm1b's weight stream. The home core computes the router (fp32) on its own tokens for the
top-2 combine weights, accumulates the 4 weighted contribution rows per
token (no base dependency), adds the base rows when they land, and writes
its 256-row output shard.
"""

import numpy as np

import concourse.bacc as bacc
import concourse.bass as bass
import concourse.mybir as mybir
import concourse.tile as tile
from concourse.bass_utils import run_bass_kernel_spmd
from concourse.masks import make_identity

P = 128
H = 2048
I_EXP = 4096
HI = I_EXP // 2             # half intermediate dim per core
E = 8
NCORE = 8
NPAIR = NCORE // 2
NTOK = 2048
TOWN = NTOK // NCORE        # 256 own token rows per core
TOB = TOWN // P             # 2 own token blocks
KO = H // P                 # 16 contraction tiles for mm1
SLE = HI // P               # 16 half-I slabs per expert section
IC_B = I_EXP // P           # 32 base chunks (full I, row-sharded base)
ICT = 2 * SLE + IC_B        # 64 gate/up slabs (a-half, b-half, base)
NB1 = 512                   # mm1 expert moving free dim (tokens)
HN = 512                    # mm2 moving free dim (H cols) = A2A quarter width
HNC = H // HN               # 4
HF = H // 2                 # column half (base mm2 lo/hi split)
NCON = 4                    # combine contributions per token (2 exp x 2 half)

F32 = mybir.dt.float32
F16 = mybir.dt.float16
I32 = mybir.dt.int32
AF = mybir.ActivationFunctionType
ALU = mybir.AluOpType
AXX = mybir.AxisListType.X

OOB_IDX = 1 << 20


def _chunks(total, step):
    out = []
    o = 0
    while o < total:
        out.append((o, min(step, total - o)))
        o += step
    return out


def _build(CA, CB, SLOT):
    NTCA = (CA + P - 1) // P
    NTCB = (CB + P - 1) // P
    NTC = NTCA + NTCB
    CAp, CBp = NTCA * P, NTCB * P
    TW_A = CA - (NTCA - 1) * P  # tokens in the last a-section block
    FLIP_A = 0 < TW_A <= 96     # flipped-orientation tail (see mm2e below)
    nc = bacc.Bacc(None)
    xeT_d = nc.dram_tensor("xeT", [P, KO, CAp + CBp], F16, kind="ExternalInput")
    xtO_d = nc.dram_tensor("xtO", [P, KO, TOWN], F16, kind="ExternalInput")
    xrO_d = nc.dram_tensor("xrO", [P, KO, TOWN], F32, kind="ExternalInput")
    wgu_d = nc.dram_tensor("wgu", [P, ICT, KO, 2 * P], F16, kind="ExternalInput")
    wde_d = nc.dram_tensor("wde", [P, 2, HNC, SLE, HN], F16, kind="ExternalInput")
    wdb_d = nc.dram_tensor("wdb", [P, HNC, IC_B, HN], F16, kind="ExternalInput")
    gw_d = nc.dram_tensor("gw", [P, KO, E], F32, kind="ExternalInput")
    dsti_d = nc.dram_tensor("dsti", [P, NTC], I32, kind="ExternalInput")
    rvi_d = nc.dram_tensor("rvi", [P, NCON * TOB], I32, kind="ExternalInput")
    cbi_d = nc.dram_tensor("cbi", [P, NCON * TOB], I32, kind="ExternalInput")
    out_d = nc.dram_tensor("out", [TOB, P, H], F16, kind="ExternalOutput")

    from contextlib import ExitStack
    with tile.TileContext(nc) as tc:
        with ExitStack() as _stk:
            def _pool(**kw):
                return _stk.enter_context(tc.tile_pool(**kw))
            persist = _pool(name="persist", bufs=1)
            xtp = _pool(name="xt", bufs=1)
            htp = _pool(name="ht", bufs=1)
            wgup = _pool(name="wgup", bufs=6)
            wdp = _pool(name="wdp", bufs=4)
            xk32p = _pool(name="xk32", bufs=1)
            tmpp = _pool(name="tmp", bufs=2)
            yesp = _pool(name="yesp", bufs=2)
            rgp = _pool(name="rgp", bufs=3)
            osbp = _pool(name="osb", bufs=1)
            rsm = _pool(name="rsm", bufs=1)
            ps1 = _pool(name="ps1", bufs=2, space="PSUM")
            ps2 = _pool(name="ps2", bufs=2, space="PSUM")
            psr = _pool(name="psr", bufs=1, space="PSUM")
            dram = _pool(name="dram", bufs=1, space="DRAM")
            send_dram = [
                dram.tile([NCORE * SLOT, HN], F16, tag=f"send{q_}", name=f"send{q_}")
                for q_ in range(HNC)
            ]
            recv_dram = [
                dram.tile([NCORE * SLOT, HN], F16, tag=f"recv{q_}", name=f"recv{q_}")
                for q_ in range(HNC)
            ]
            comb_dram = dram.tile([TOWN * E, 1], F32, tag="combd")

            # ===== mm1 expert: gate/up + silu*up on both half-experts ======
            # xeT columns: [pair-expert-a tokens | pad | expert-b tokens |
            # pad]; pads are zero so h comes out 0 and the rows map to OOB
            # send slots downstream. xeT arrives in k-chunks so the first
            # matmuls start as soon as chunk 0 lands.
            xeT = xtp.tile([P, KO, CAp + CBp], F16, tag="xt", name="xeT")
            for kq in range(4):
                nc.sync.dma_start(
                    xeT[:, kq * (KO // 4):(kq + 1) * (KO // 4), :],
                    xeT_d[:, kq * (KO // 4):(kq + 1) * (KO // 4)],
                )
            gw_sb = persist.tile([P, KO, E], F32, tag="gw")
            nc.sync.dma_start(gw_sb, gw_d[:])
            dsti_sb = persist.tile([P, NTC], I32, tag="dsti")
            nc.sync.dma_start(dsti_sb, dsti_d[:])
            rvi_sb = persist.tile([P, NCON * TOB], I32, tag="rvi")
            nc.sync.dma_start(rvi_sb, rvi_d[:])
            cbi_sb = persist.tile([P, NCON * TOB], I32, tag="cbi")
            nc.sync.dma_start(cbi_sb, cbi_d[:])
            if FLIP_A:
                ident = persist.tile([P, P], F16, tag="ident")
                make_identity(nc, ident)
            ht_e = htp.tile([P, SLE, CAp + CBp], F16, tag="hte")
            for sect in range(2):
                Cs = CA if sect == 0 else CB
                coff = 0 if sect == 0 else CAp
                for i in range(SLE):
                    slab = wgup.tile(
                        [P, KO, 2 * P], F16, tag="slab", name=f"sl{sect}_{i}"
                    )
                    nc.sync.dma_start(slab, wgu_d[:, sect * SLE + i])
                    for (no, nw) in _chunks(Cs, NB1):
                        nsl = slice(coff + no, coff + no + nw)
                        pg = ps1.tile([P, NB1], F32, tag="pg", name=f"pg{sect}_{i}_{no}")
                        pu = ps1.tile([P, NB1], F32, tag="pu", name=f"pu{sect}_{i}_{no}")
                        for k in range(KO):
                            nc.tensor.matmul(
                                pg[:, :nw], slab[:, k, 0:P], xeT[:, k, nsl],
                                start=(k == 0), stop=(k == KO - 1),
                            )
                        for k in range(KO):
                            nc.tensor.matmul(
                                pu[:, :nw], slab[:, k, P:2 * P], xeT[:, k, nsl],
                                start=(k == 0), stop=(k == KO - 1),
                            )
                        sil = tmpp.tile([P, NB1], F16, tag="sil")
                        nc.scalar.activation(sil[:, :nw], pg[:, :nw], AF.Silu)
                        nc.vector.tensor_tensor(
                            ht_e[:, i, nsl], sil[:, :nw], pu[:, :nw], ALU.mult
                        )

            # ===== mm2 expert (half-I down partials) on dispatched tokens ==
            # One H-quarter per cc chunk: scatter each block's rows as they
            # finish and fire that quarter's AllToAll immediately, so all
            # collective traffic drains during mm2e (light DMA load) instead
            # of colliding with mm1b's weight streaming.
            for cc in range(HNC):
                subs = []
                for sect in range(2):
                    ss = []
                    for sub in range(2):
                        w = wdp.tile(
                            [P, SLE // 2, HN], F16, tag="wsl",
                            name=f"we{cc}_{sect}_{sub}",
                        )
                        nc.sync.dma_start(
                            w,
                            wde_d[:, sect, cc,
                                  sub * (SLE // 2):(sub + 1) * (SLE // 2)],
                        )
                        ss.append(w)
                    subs.append(ss)
                yesq = yesp.tile([P, NTC, HN], F16, tag="yes", name=f"yes{cc}")
                for bi in range(NTC):
                    sect = 0 if bi < NTCA else 1
                    col = bi * P if sect == 0 else CAp + (bi - NTCA) * P
                    if FLIP_A and bi == NTCA - 1:
                        # mostly-empty a-tail block: flipped orientation
                        # (tokens moving) costs ~16*TW cycles per H-block
                        # instead of 512 per accumulation chunk, then a PE
                        # transpose restores token-major rows for the
                        # scatter. Weights come from the already-loaded
                        # subs tiles (no extra DMA).
                        for hq in range(HN // P):
                            pyt = ps2.tile(
                                [P, P], F32, tag="py", name=f"pyt{cc}_{hq}"
                            )
                            for k in range(SLE):
                                st = subs[0][k // (SLE // 2)][
                                    :, k % (SLE // 2), hq * P:(hq + 1) * P
                                ]
                                nc.tensor.matmul(
                                    pyt[:, :TW_A], st,
                                    ht_e[:, k, col:col + TW_A],
                                    start=(k == 0), stop=(k == SLE - 1),
                                )
                            ytT = tmpp.tile([P, P], F16, tag="ytT")
                            nc.scalar.activation(
                                ytT[:, :TW_A], pyt[:, :TW_A], AF.Copy
                            )
                            ptr = psr.tile(
                                [P, P], F16, tag="ptr", name=f"ptr{cc}_{hq}"
                            )
                            nc.tensor.transpose(
                                ptr[:TW_A, :], ytT[:, :TW_A], ident
                            )
                            nc.scalar.activation(
                                yesq[:TW_A, bi, hq * P:(hq + 1) * P],
                                ptr[:TW_A, :], AF.Copy,
                            )
                    else:
                        py = ps2.tile([P, HN], F32, tag="py", name=f"pye{cc}_{bi}")
                        for k in range(SLE):
                            nc.tensor.matmul(
                                py, ht_e[:, k, col:col + P],
                                subs[sect][k // (SLE // 2)][:, k % (SLE // 2), :],
                                start=(k == 0), stop=(k == SLE - 1),
                            )
                        nc.scalar.activation(yesq[:, bi, :], py, AF.Copy)
                    nc.gpsimd.indirect_dma_start(
                        out=send_dram[cc][:],
                        out_offset=bass.IndirectOffsetOnAxis(
                            ap=dsti_sb[:, bi:bi + 1], axis=0
                        ),
                        in_=yesq[:, bi, :],
                        in_offset=None,
                        bounds_check=NCORE * SLOT - 1,
                        oob_is_err=False,
                    )
                nc.gpsimd.collective_compute(
                    "AllToAll",
                    ALU.bypass,
                    replica_groups=[list(range(NCORE))],
                    ins=[send_dram[cc][:]],
                    outs=[recv_dram[cc][:]],
                )

            # ============ router on own 256 tokens (strict fp32) ===========
            # logits^T: stationary = own x^T block [128h, 128tok], moving =
            # gw [128h, 8]; accumulate over k. One accumulation group at a
            # time per PSUM bank (start=True clears the whole bank's bits).
            zl_ps = psr.tile([P, TOB, E], F32, tag="zlps")
            for tb in range(TOB):
                xk = xk32p.tile([P, KO, P], F32, tag="xk")
                nc.sync.dma_start(xk, xrO_d[:, :, tb * P:(tb + 1) * P])
                for k in range(KO):
                    nc.tensor.matmul(
                        zl_ps[:, tb, :],
                        xk[:, k, :],
                        gw_sb[:, k, :],
                        start=(k == 0), stop=(k == KO - 1),
                    )
            zl = rsm.tile([P, TOB, E], F32, tag="zl")
            nc.vector.tensor_copy(zl, zl_ps)
            lmax = rsm.tile([P, TOB], F32, tag="lmax")
            nc.vector.reduce_max(lmax[:, :, None], zl, axis=AXX)
            nmax = rsm.tile([P, TOB], F32, tag="nmax")
            nc.vector.tensor_scalar_mul(nmax, lmax, -1.0)
            zex = rsm.tile([P, TOB, E], F32, tag="zex")
            for tb in range(TOB):
                nc.scalar.activation(
                    zex[:, tb, :], zl[:, tb, :], AF.Exp, bias=nmax[:, tb:tb + 1]
                )
            zlt = rsm.tile([P, TOB, E], F32, tag="zlt")
            nc.vector.tensor_scalar(zlt, zex, 1.0, None, op0=ALU.is_lt)
            zmk = rsm.tile([P, TOB, E], F32, tag="zmk")
            nc.vector.tensor_tensor(zmk, zex, zlt, ALU.mult)
            m2 = rsm.tile([P, TOB], F32, tag="m2")
            nc.vector.reduce_max(m2[:, :, None], zmk, axis=AXX)
            # per-expert top-2 mask and normalized weights: w_e =
            # zex_e * [zex_e >= m2] / (1 + m2)
            ge = rsm.tile([P, TOB, E], F32, tag="ge")
            nc.vector.tensor_tensor(
                ge, zex, m2[:, :, None].to_broadcast((P, TOB, E)), ALU.is_ge
            )
            s1 = rsm.tile([P, TOB], F32, tag="s1")
            nc.vector.tensor_scalar_add(s1, m2, 1.0)
            rcp = rsm.tile([P, TOB], F32, tag="rcp")
            nc.vector.reciprocal(rcp, s1)
            cw = rsm.tile([P, TOB, E], F32, tag="cw")
            nc.vector.tensor_tensor(cw, zex, ge, ALU.mult)
            cwn = rsm.tile([P, TOB, E], F32, tag="cwn")
            nc.vector.tensor_tensor(
                cwn, cw, rcp[:, :, None].to_broadcast((P, TOB, E)), ALU.mult
            )
            # store [TOWN*E, 1] with flat index (tb*128 + p)*8 + e
            nc.sync.dma_start(
                comb_dram[:].rearrange(
                    "(b p e) one -> p b (e one)", p=P, b=TOB, e=E
                ),
                cwn,
            )

            # prefetch combine-weight rows (router output, ready long ago)
            # before the gpsimd queue blocks on the collective
            cbs = []
            for sidx in range(NCON * TOB):
                cb = rgp.tile([P, 1], F32, tag=f"cb{sidx}", name=f"cb{sidx}")
                nc.gpsimd.indirect_dma_start(
                    out=cb[:],
                    out_offset=None,
                    in_=comb_dram[:],
                    in_offset=bass.IndirectOffsetOnAxis(
                        ap=cbi_sb[:, sidx:sidx + 1], axis=0
                    ),
                    bounds_check=TOWN * E - 1,
                    oob_is_err=False,
                )
                cbs.append(cb)
            # ============ mm1 base: own 256 tokens, full I =================
            xtO = xtp.tile([P, KO, TOWN], F16, tag="xt", name="xtO")
            nc.sync.dma_start(xtO, xtO_d[:])
            ht_b = htp.tile([P, IC_B, TOWN], F16, tag="hte", name="ht_b")
            for j in range(IC_B):
                slab = wgup.tile([P, KO, 2 * P], F16, tag="slab", name=f"slb{j}")
                slab_dma = nc.sync.dma_start(slab, wgu_d[:, 2 * SLE + j])
                if j == IC_B - 1:
                    last_mm1b_slab_dma = slab_dma
                pg = ps1.tile([P, TOWN], F32, tag="pg", name=f"bpg{j}")
                pu = ps1.tile([P, TOWN], F32, tag="pu", name=f"bpu{j}")
                # interleave gate/up so each LDWEIGHTS hides under the
                # previous matmul (N=256 leaves no slack otherwise)
                for k in range(KO):
                    nc.tensor.matmul(
                        pg, slab[:, k, 0:P], xtO[:, k, :],
                        start=(k == 0), stop=(k == KO - 1),
                    )
                    nc.tensor.matmul(
                        pu, slab[:, k, P:2 * P], xtO[:, k, :],
                        start=(k == 0), stop=(k == KO - 1),
                    )
                sil = tmpp.tile([P, TOWN], F16, tag="sil")
                nc.scalar.activation(sil, pg, AF.Silu)
                nc.vector.tensor_tensor(ht_b[:, j, :], sil, pu, ALU.mult)
                if j == 22:
                    # prefetch mm2b's first column chunk of base down weights
                    # here: emitted later they would queue behind the rest of
                    # the slab DMAs and stall mm2b's first matmuls
                    wdb0 = []
                    for ss in range(IC_B // (SLE // 2)):
                        w = wdp.tile(
                            [P, SLE // 2, HN], F16, tag="wsl", name=f"wb0_{ss}"
                        )
                        nc.sync.dma_start(
                            w,
                            wdb_d[:, 0, ss * (SLE // 2):(ss + 1) * (SLE // 2)],
                        )
                        wdb0.append(w)

            # ===== receive: gather 4 partial rows per token, accumulate ====
            # The weighted expert accumulation has no base dependency, so it
            # runs on the vector engine underneath the base down-projection.
            accs = [
                osbp.tile([P, H], F16, tag=f"osb{tb_}", name=f"osb{tb_}")
                for tb_ in range(TOB)
            ]
            for q in range(HNC):
                qsl = slice(q * HN, (q + 1) * HN)
                for tb in range(TOB):
                    for j in range(NCON):
                        sidx = j * TOB + tb
                        rg = rgp.tile([P, HN], F16, tag="rg")
                        rg_dma = nc.gpsimd.indirect_dma_start(
                            out=rg[:],
                            out_offset=None,
                            in_=recv_dram[q][:],
                            in_offset=bass.IndirectOffsetOnAxis(
                                ap=rvi_sb[:, sidx:sidx + 1], axis=0
                            ),
                            bounds_check=NCORE * SLOT - 1,
                            oob_is_err=False,
                        )
                        # Pin behind mm1b's last weight load: issued earlier,
                        # this gather's A2A-completion wait head-of-line
                        # blocks later DMAs sharing its completion lane.
                        bass._add_dep_helper(
                            rg_dma.ins, last_mm1b_slab_dma.ins, sync=True,
                            reason="defer recv gather",
                        )
                        if j == 0:
                            nc.vector.tensor_scalar_mul(
                                accs[tb][:, qsl], rg[:], cbs[sidx][:]
                            )
                        else:
                            nc.vector.scalar_tensor_tensor(
                                accs[tb][:, qsl], rg[:], cbs[sidx][:],
                                accs[tb][:, qsl], ALU.mult, ALU.add,
                            )

            # ============ mm2 base (down) on own tokens ====================
            base_lo = yesp.tile([P, TOB, HF], F16, tag="yes", name="base_lo")
            base_hi = xk32p.tile([P, TOB, HF], F16, tag="xk", name="base_hi")
            for cc in range(HNC):
                nsub = IC_B // (SLE // 2)
                if cc == 0:
                    bsubs = wdb0
                else:
                    bsubs = []
                    for ss in range(nsub):
                        w = wdp.tile(
                            [P, SLE // 2, HN], F16, tag="wsl", name=f"wb{cc}_{ss}"
                        )
                        nc.sync.dma_start(
                            w, wdb_d[:, cc, ss * (SLE // 2):(ss + 1) * (SLE // 2)]
                        )
                        bsubs.append(w)
                for tb in range(TOB):
                    py = ps2.tile([P, HN], F32, tag="py", name=f"pyb{cc}_{tb}")
                    for j in range(IC_B):
                        nc.tensor.matmul(
                            py, ht_b[:, j, tb * P:(tb + 1) * P],
                            bsubs[j // (SLE // 2)][:, j % (SLE // 2), :],
                            start=(j == 0), stop=(j == IC_B - 1),
                        )
                    bdst = base_lo if cc < HNC // 2 else base_hi
                    bcc = cc % (HNC // 2)
                    nc.scalar.activation(
                        bdst[:, tb, bcc * HN:(bcc + 1) * HN], py, AF.Copy
                    )

            # ====== add base rows; low half first so it hides under the ====
            # ====== remaining base down-projection; write output shard =====
            for half in range(2):
                hsl = slice(half * HF, (half + 1) * HF)
                base_h = (base_lo, base_hi)[half]
                for tb in range(TOB):
                    nc.vector.tensor_tensor(
                        accs[tb][:, hsl], accs[tb][:, hsl], base_h[:, tb, :],
                        ALU.add,
                    )
            for tb in range(TOB):
                nc.sync.dma_start(out_d[tb], accs[tb])

    return nc


def _prep_inputs(x, gate_w, base_gate_up, base_down, expert_gate_up, expert_down):
    xf = np.ascontiguousarray(np.asarray(x, np.float32).reshape(NTOK, H))
    xT = np.ascontiguousarray(xf.reshape(NTOK, KO, P).transpose(2, 1, 0))
    xt16 = xT.astype(np.float16)
    gwf = np.asarray(gate_w, np.float32)
    gwp = np.ascontiguousarray(gwf.reshape(KO, P, E).transpose(1, 0, 2))

    # host-side dispatch: which tokens go to which expert (top-2 of logits)
    logits = xf @ gwf
    order = np.argsort(-logits, axis=1)
    top2 = order[:, :2]
    sel = [np.where((top2 == c).any(axis=1))[0].astype(np.int64) for c in range(E)]
    counts = np.array([len(s) for s in sel])

    # pair heavy experts with light ones so pair token sums are near-equal;
    # cores 2p / 2p+1 hold the low / high I-halves of pair p's two experts
    od = np.argsort(-counts, kind="stable")
    pairs = [(int(od[i]), int(od[E - 1 - i])) for i in range(NPAIR)]
    e2pr = {}
    for pi, (a, b) in enumerate(pairs):
        e2pr[a] = (pi, 0)
        e2pr[b] = (pi, 1)

    CA = int(max(counts[a] for a, b in pairs))
    CB = int(max(counts[b] for a, b in pairs))
    NTCA = (CA + P - 1) // P
    NTCB = (CB + P - 1) // P
    CAp, CBp = NTCA * P, NTCB * P
    NTC = NTCA + NTCB

    # per-pair concatenated token columns: [a tokens|pad] + [b tokens|pad];
    # send position = order of appearance within the (pair -> home) group
    pair_flat = []
    pair_pos = []
    pair_colof = []
    max_grp = 0
    for pi, (a, b) in enumerate(pairs):
        La, Lb = sel[a], sel[b]
        flat = np.full(CAp + CBp, -1, np.int64)
        flat[: len(La)] = La
        flat[CAp:CAp + len(Lb)] = Lb
        real = np.nonzero(flat >= 0)[0]
        pos = np.full(CAp + CBp, OOB_IDX, np.int64)
        cnt = np.zeros(NCORE, np.int64)
        colof = {}
        for ci in real:
            hm = flat[ci] // TOWN
            pos[ci] = cnt[hm]
            cnt[hm] += 1
            colof[(int(flat[ci]), 0 if ci < CAp else 1)] = ci
        max_grp = max(max_grp, int(cnt.max()))
        pair_flat.append(flat)
        pair_pos.append(pos)
        pair_colof.append(colof)
    SLOT = (max_grp + 3) // 4 * 4

    # per-pair send index: column (block bi, partition p) -> home*SLOT + pos
    dsti_p = []
    for pi in range(NPAIR):
        flat, pos = pair_flat[pi], pair_pos[pi]
        dst = np.where(
            flat >= 0, (flat // TOWN) * SLOT + pos, OOB_IDX
        ).astype(np.int64)
        dsti_p.append(
            np.ascontiguousarray(dst.reshape(NTC, P).T.astype(np.int32))
        )

    # per-core receive index: own token t, contribution j in 0..3 =
    # (expert rank j//2 sorted, I-half j%2): recv row = sender*SLOT + pos
    rvi = np.zeros((NCORE, P, NCON * TOB), np.int32)
    cbi = np.zeros((NCORE, P, NCON * TOB), np.int32)
    for hme in range(NCORE):
        for tl in range(TOWN):
            t = hme * TOWN + tl
            tb, p = divmod(tl, P)
            es = np.sort(top2[t])
            for j2, e in enumerate(es):
                pi, role = e2pr[int(e)]
                ci = pair_colof[pi][(t, role)]
                ps_ = int(pair_pos[pi][ci])
                for half in range(2):
                    j = j2 * 2 + half
                    sender = 2 * pi + half
                    rvi[hme, p, j * TOB + tb] = sender * SLOT + ps_
                    cbi[hme, p, j * TOB + tb] = tl * E + e

    bgu = np.asarray(base_gate_up, np.float32)
    gb_ = bgu[:, :I_EXP].reshape(H, IC_B, P)
    ub_ = bgu[:, I_EXP:].reshape(H, IC_B, P)
    pb_ = np.concatenate([gb_, ub_], axis=2)  # [H, IC_B, 2P]
    bd = np.asarray(base_down, np.float32)
    wdb_p = np.ascontiguousarray(
        bd.reshape(IC_B, P, HNC, HN).transpose(1, 2, 0, 3)
    ).astype(np.float16)

    in_maps = []
    for c in range(NCORE):
        pi, half = divmod(c, 2)
        a, b = pairs[pi]

        def half_slabs(We):
            We = np.asarray(We, np.float32)
            g = We[:, half * HI:(half + 1) * HI].reshape(H, SLE, P)
            u = We[:, I_EXP + half * HI:I_EXP + (half + 1) * HI].reshape(
                H, SLE, P
            )
            return np.concatenate([g, u], axis=2)  # [H, SLE, 2P]

        allp = np.concatenate(
            [half_slabs(expert_gate_up[a]), half_slabs(expert_gate_up[b]), pb_],
            axis=1,
        )  # [H, ICT, 2P]
        wgu_p = np.ascontiguousarray(
            allp.reshape(KO, P, ICT, 2 * P).transpose(1, 2, 0, 3)
        ).astype(np.float16)

        def half_down(ed_):
            d = np.asarray(ed_, np.float32)[half * HI:(half + 1) * HI]
            return d.reshape(SLE, P, HNC, HN).transpose(1, 2, 0, 3)

        wde_p = np.ascontiguousarray(
            np.stack(
                [half_down(expert_down[a]), half_down(expert_down[b])], axis=1
            )
        ).astype(np.float16)  # [P, 2, HNC, SLE, HN]

        flat = pair_flat[pi]
        real = flat >= 0
        xe = np.zeros((P, KO, CAp + CBp), np.float16)
        xe[:, :, real] = xt16[:, :, flat[real]]
        own = slice(c * TOWN, (c + 1) * TOWN)
        in_maps.append(
            dict(
                xeT=np.ascontiguousarray(xe),
                xtO=np.ascontiguousarray(xt16[:, :, own]),
                xrO=np.ascontiguousarray(xT[:, :, own]),
                wgu=wgu_p, wde=wde_p, wdb=wdb_p, gw=gwp,
                dsti=dsti_p[pi], rvi=rvi[c], cbi=cbi[c],
            )
        )
    return in_maps, CA, CB, SLOT


LAST_RESULTS = None


def kernel(x, gate_w, base_gate_up, base_down, expert_gate_up, expert_down):
    global LAST_RESULTS
    in_maps, CA, CB, SLOT = _prep_inputs(
        x, gate_w, base_gate_up, base_down, expert_gate_up, expert_down
    )
    nc = _build(CA, CB, SLOT)
    if not nc.is_finalized():
        nc.finalize()
    res = run_bass_kernel_spmd(nc, in_maps, core_ids=list(range(NCORE)))
    LAST_RESULTS = res
    y = np.empty((NTOK, H), np.float32)
    for c in range(NCORE):
        o = res.results[c]["out"]  # [TOB, P, H] f16
        y[c * TOWN:(c + 1) * TOWN] = o.reshape(TOWN, H).astype(np.float32)
    return y.reshape(1, NTOK, H)


if __name__ == "__main__":
    nc = _build(545, 510, 160)
    print("build ok; instructions:",
          sum(len(b.instructions) for b in nc.main_func.blocks))


# revision 17
# speedup vs baseline: 1.0975x; 1.0046x over previous
"""LlamaMoE (H=2048, I=4096, E=8 experts, top-2, N=2048 tokens) on 8 trn2 cores.

Strategy: PAIR-SPLIT expert parallelism + token-parallel base MLP, combined
with a single split AllToAll.

The tensor engine is power-throttled to ~1.9 GHz sustained and the baseline
schedule had zero tensor idle, so the only win is fewer matmul cycles.
Expert token counts are imbalanced (484..545 vs 512 avg); with one expert
per core every core pays for the worst count. Instead, experts are PAIRED
large-with-small (sums 1019..1029, +-0.5%): the two cores of a pair each
hold HALF of the intermediate dim (I/2) of BOTH experts and process the
pair's full token list. Each (token, expert) down-projection row is then a
half-I partial computed on two cores; both partials ride the AllToAll and
the home core sums 4 contributions (2 experts x 2 halves) instead of 2.
Per-core matmul cycles drop ~6% and are balanced regardless of routing
skew, with identical weight DMA volume.

Host supplies the dispatch permutation (pre-gathered transposed activations
plus send/receive index maps padded with OOB sentinels); all model math --
router logits, top-2 combine weights, expert MLPs, base MLP, combine --
runs on device. The base MLP stays row-sharded: core c computes the full
base MLP for its own 256 token rows (no cross-core reduction).

Expert partial rows are scattered into an AllToAll send buffer grouped by
destination (token-home) core; one fp16 AllToAll per H-quarter fires as
soon as that column chunk of the down-projection completes, so all four
collectives drain during mm2e itself (light DMA phase) instead of starving
mm1b's weight stream. The home core computes the router (fp32) on its own tokens for the
top-2 combine weights, accumulates the 4 weighted contribution rows per
token (no base dependency), adds the base rows when they land, and writes
its 256-row output shard.
"""

import numpy as np

import concourse.bacc as bacc
import concourse.bass as bass
import concourse.mybir as mybir
import concourse.tile as tile
from concourse.bass_utils import run_bass_kernel_spmd
from concourse.masks import make_identity

P = 128
H = 2048
I_EXP = 4096
HI = I_EXP // 2             # half intermediate dim per core
E = 8
NCORE = 8
NPAIR = NCORE // 2
NTOK = 2048
TOWN = NTOK // NCORE        # 256 own token rows per core
TOB = TOWN // P             # 2 own token blocks
KO = H // P                 # 16 contraction tiles for mm1
SLE = HI // P               # 16 half-I slabs per expert section
IC_B = I_EXP // P           # 32 base chunks (full I, row-sharded base)
ICT = 2 * SLE + IC_B        # 64 gate/up slabs (a-half, b-half, base)
NB1 = 512                   # mm1 expert moving free dim (tokens)
HN = 512                    # mm2 moving free dim (H cols) = A2A quarter width
HNC = H // HN               # 4
HF = H // 2                 # column half (base mm2 lo/hi split)
NCON = 4                    # combine contributions per token (2 exp x 2 half)

F32 = mybir.dt.float32
F16 = mybir.dt.float16
I32 = mybir.dt.int32
AF = mybir.ActivationFunctionType
ALU = mybir.AluOpType
AXX = mybir.AxisListType.X

OOB_IDX = 1 << 20


def _chunks(total, step):
    out = []
    o = 0
    while o < total:
        out.append((o, min(step, total - o)))
        o += step
    return out


def _build(CA, CB, SLOT):
    NTCA = (CA + P - 1) // P
    NTCB = (CB + P - 1) // P
    NTC = NTCA + NTCB
    CAp, CBp = NTCA * P, NTCB * P
    TW_A = CA - (NTCA - 1) * P  # tokens in the last a-section block
    FLIP_A = 0 < TW_A <= 96     # flipped-orientation tail (see mm2e below)
    nc = bacc.Bacc(None)
    xeT_d = nc.dram_tensor("xeT", [P, KO, CAp + CBp], F16, kind="ExternalInput")
    xtO_d = nc.dram_tensor("xtO", [P, KO, TOWN], F16, kind="ExternalInput")
    xrO_d = nc.dram_tensor("xrO", [P, KO, TOWN], F32, kind="ExternalInput")
    wgu_d = nc.dram_tensor("wgu", [P, ICT, KO, 2 * P], F16, kind="ExternalInput")
    wde_d = nc.dram_tensor("wde", [P, 2, HNC, SLE, HN], F16, kind="ExternalInput")
    wdb_d = nc.dram_tensor("wdb", [P, HNC, IC_B, HN], F16, kind="ExternalInput")
    gw_d = nc.dram_tensor("gw", [P, KO, E], F32, kind="ExternalInput")
    dsti_d = nc.dram_tensor("dsti", [P, NTC], I32, kind="ExternalInput")
    rvi_d = nc.dram_tensor("rvi", [P, NCON * TOB], I32, kind="ExternalInput")
    cbi_d = nc.dram_tensor("cbi", [P, NCON * TOB], I32, kind="ExternalInput")
    out_d = nc.dram_tensor("out", [TOB, P, H], F16, kind="ExternalOutput")

    from contextlib import ExitStack
    with tile.TileContext(nc) as tc:
        with ExitStack() as _stk:
            def _pool(**kw):
                return _stk.enter_context(tc.tile_pool(**kw))
            persist = _pool(name="persist", bufs=1)
            xtp = _pool(name="xt", bufs=1)
            htp = _pool(name="ht", bufs=1)
            wgup = _pool(name="wgup", bufs=6)
            wdp = _pool(name="wdp", bufs=4)
            xk32p = _pool(name="xk32", bufs=1)
            tmpp = _pool(name="tmp", bufs=2)
            yesp = _pool(name="yesp", bufs=2)
            rgp = _pool(name="rgp", bufs=3)
            osbp = _pool(name="osb", bufs=1)
            rsm = _pool(name="rsm", bufs=1)
            ps1 = _pool(name="ps1", bufs=2, space="PSUM")
            ps2 = _pool(name="ps2", bufs=2, space="PSUM")
            psr = _pool(name="psr", bufs=1, space="PSUM")
            dram = _pool(name="dram", bufs=1, space="DRAM")
            send_dram = [
                dram.tile([NCORE * SLOT, HN], F16, tag=f"send{q_}", name=f"send{q_}")
                for q_ in range(HNC)
            ]
            recv_dram = [
                dram.tile([NCORE * SLOT, HN], F16, tag=f"recv{q_}", name=f"recv{q_}")
                for q_ in range(HNC)
            ]
            comb_dram = dram.tile([TOWN * E, 1], F32, tag="combd")

            # ===== mm1 expert: gate/up + silu*up on both half-experts ======
            # xeT columns: [pair-expert-a tokens | pad | expert-b tokens |
            # pad]; pads are zero so h comes out 0 and the rows map to OOB
            # send slots downstream. xeT arrives in k-chunks so the first
            # matmuls start as soon as chunk 0 lands.
            xeT = xtp.tile([P, KO, CAp + CBp], F16, tag="xt", name="xeT")
            for kq in range(4):
                nc.sync.dma_start(
                    xeT[:, kq * (KO // 4):(kq + 1) * (KO // 4), :],
                    xeT_d[:, kq * (KO // 4):(kq + 1) * (KO // 4)],
                )
            gw_sb = persist.tile([P, KO, E], F32, tag="gw")
            nc.sync.dma_start(gw_sb, gw_d[:])
            dsti_sb = persist.tile([P, NTC], I32, tag="dsti")
            nc.sync.dma_start(dsti_sb, dsti_d[:])
            rvi_sb = persist.tile([P, NCON * TOB], I32, tag="rvi")
            nc.sync.dma_start(rvi_sb, rvi_d[:])
            cbi_sb = persist.tile([P, NCON * TOB], I32, tag="cbi")
            nc.sync.dma_start(cbi_sb, cbi_d[:])
            if FLIP_A:
                ident = persist.tile([P, P], F16, tag="ident")
                make_identity(nc, ident)
            ht_e = htp.tile([P, SLE, CAp + CBp], F16, tag="hte")
            for sect in range(2):
                Cs = CA if sect == 0 else CB
                coff = 0 if sect == 0 else CAp
                for i in range(SLE):
                    slab = wgup.tile(
                        [P, KO, 2 * P], F16, tag="slab", name=f"sl{sect}_{i}"
                    )
                    nc.sync.dma_start(slab, wgu_d[:, sect * SLE + i])
                    for (no, nw) in _chunks(Cs, NB1):
                        nsl = slice(coff + no, coff + no + nw)
                        pg = ps1.tile([P, NB1], F32, tag="pg", name=f"pg{sect}_{i}_{no}")
                        pu = ps1.tile([P, NB1], F32, tag="pu", name=f"pu{sect}_{i}_{no}")
                        for k in range(KO):
                            nc.tensor.matmul(
                                pg[:, :nw], slab[:, k, 0:P], xeT[:, k, nsl],
                                start=(k == 0), stop=(k == KO - 1),
                            )
                        for k in range(KO):
                            nc.tensor.matmul(
                                pu[:, :nw], slab[:, k, P:2 * P], xeT[:, k, nsl],
                                start=(k == 0), stop=(k == KO - 1),
                            )
                        sil = tmpp.tile([P, NB1], F16, tag="sil")
                        nc.scalar.activation(sil[:, :nw], pg[:, :nw], AF.Silu)
                        nc.vector.tensor_tensor(
                            ht_e[:, i, nsl], sil[:, :nw], pu[:, :nw], ALU.mult
                        )

            # ===== mm2 expert (half-I down partials) on dispatched tokens ==
            # One H-quarter per cc chunk: scatter each block's rows as they
            # finish and fire that quarter's AllToAll immediately, so all
            # collective traffic drains during mm2e (light DMA load) instead
            # of colliding with mm1b's weight streaming.
            for cc in range(HNC):
                subs = []
                for sect in range(2):
                    ss = []
                    for sub in range(2):
                        w = wdp.tile(
                            [P, SLE // 2, HN], F16, tag="wsl",
                            name=f"we{cc}_{sect}_{sub}",
                        )
                        nc.sync.dma_start(
                            w,
                            wde_d[:, sect, cc,
                                  sub * (SLE // 2):(sub + 1) * (SLE // 2)],
                        )
                        ss.append(w)
                    subs.append(ss)
                yesq = yesp.tile([P, NTC, HN], F16, tag="yes", name=f"yes{cc}")
                ytTs = []
                for bi in range(NTC):
                    sect = 0 if bi < NTCA else 1
                    col = bi * P if sect == 0 else CAp + (bi - NTCA) * P
                    if FLIP_A and bi == NTCA - 1:
                        # mostly-empty a-tail block: flipped orientation
                        # (tokens moving) costs ~16*TW cycles per H-block
                        # instead of 512 per accumulation chunk; a PE
                        # transpose restores token-major rows for the
                        # scatter. Weights come from the already-loaded
                        # subs tiles (no extra DMA). The transposes are
                        # deferred past the b-section blocks so their input
                        # copies never stall the PE.
                        for hq in range(HN // P):
                            pyt = ps2.tile(
                                [P, P], F32, tag="py", name=f"pyt{cc}_{hq}"
                            )
                            for k in range(SLE):
                                st = subs[0][k // (SLE // 2)][
                                    :, k % (SLE // 2), hq * P:(hq + 1) * P
                                ]
                                nc.tensor.matmul(
                                    pyt[:, :TW_A], st,
                                    ht_e[:, k, col:col + TW_A],
                                    start=(k == 0), stop=(k == SLE - 1),
                                )
                            ytT = tmpp.tile([P, P], F16, tag=f"ytT{hq}")
                            nc.scalar.activation(
                                ytT[:, :TW_A], pyt[:, :TW_A], AF.Copy
                            )
                            ytTs.append(ytT)
                        continue
                    py = ps2.tile([P, HN], F32, tag="py", name=f"pye{cc}_{bi}")
                    for k in range(SLE):
                        nc.tensor.matmul(
                            py, ht_e[:, k, col:col + P],
                            subs[sect][k // (SLE // 2)][:, k % (SLE // 2), :],
                            start=(k == 0), stop=(k == SLE - 1),
                        )
                    nc.scalar.activation(yesq[:, bi, :], py, AF.Copy)
                    nc.gpsimd.indirect_dma_start(
                        out=send_dram[cc][:],
                        out_offset=bass.IndirectOffsetOnAxis(
                            ap=dsti_sb[:, bi:bi + 1], axis=0
                        ),
                        in_=yesq[:, bi, :],
                        in_offset=None,
                        bounds_check=NCORE * SLOT - 1,
                        oob_is_err=False,
                    )
                if FLIP_A:
                    bi = NTCA - 1
                    for hq in range(HN // P):
                        ptr = psr.tile(
                            [P, P], F16, tag="ptr", name=f"ptr{cc}_{hq}"
                        )
                        nc.tensor.transpose(
                            ptr[:TW_A, :], ytTs[hq][:, :TW_A], ident
                        )
                        nc.scalar.activation(
                            yesq[:TW_A, bi, hq * P:(hq + 1) * P],
                            ptr[:TW_A, :], AF.Copy,
                        )
                    nc.gpsimd.indirect_dma_start(
                        out=send_dram[cc][:],
                        out_offset=bass.IndirectOffsetOnAxis(
                            ap=dsti_sb[:, bi:bi + 1], axis=0
                        ),
                        in_=yesq[:, bi, :],
                        in_offset=None,
                        bounds_check=NCORE * SLOT - 1,
                        oob_is_err=False,
                    )
                nc.gpsimd.collective_compute(
                    "AllToAll",
                    ALU.bypass,
                    replica_groups=[list(range(NCORE))],
                    ins=[send_dram[cc][:]],
                    outs=[recv_dram[cc][:]],
                )

            # ============ router on own 256 tokens (strict fp32) ===========
            # logits^T: stationary = own x^T block [128h, 128tok], moving =
            # gw [128h, 8]; accumulate over k. One accumulation group at a
            # time per PSUM bank (start=True clears the whole bank's bits).
            zl_ps = psr.tile([P, TOB, E], F32, tag="zlps")
            for tb in range(TOB):
                xk = xk32p.tile([P, KO, P], F32, tag="xk")
                nc.sync.dma_start(xk, xrO_d[:, :, tb * P:(tb + 1) * P])
                for k in range(KO):
                    nc.tensor.matmul(
                        zl_ps[:, tb, :],
                        xk[:, k, :],
                        gw_sb[:, k, :],
                        start=(k == 0), stop=(k == KO - 1),
                    )
            zl = rsm.tile([P, TOB, E], F32, tag="zl")
            nc.vector.tensor_copy(zl, zl_ps)
            lmax = rsm.tile([P, TOB], F32, tag="lmax")
            nc.vector.reduce_max(lmax[:, :, None], zl, axis=AXX)
            nmax = rsm.tile([P, TOB], F32, tag="nmax")
            nc.vector.tensor_scalar_mul(nmax, lmax, -1.0)
            zex = rsm.tile([P, TOB, E], F32, tag="zex")
            for tb in range(TOB):
                nc.scalar.activation(
                    zex[:, tb, :], zl[:, tb, :], AF.Exp, bias=nmax[:, tb:tb + 1]
                )
            zlt = rsm.tile([P, TOB, E], F32, tag="zlt")
            nc.vector.tensor_scalar(zlt, zex, 1.0, None, op0=ALU.is_lt)
            zmk = rsm.tile([P, TOB, E], F32, tag="zmk")
            nc.vector.tensor_tensor(zmk, zex, zlt, ALU.mult)
            m2 = rsm.tile([P, TOB], F32, tag="m2")
            nc.vector.reduce_max(m2[:, :, None], zmk, axis=AXX)
            # per-expert top-2 mask and normalized weights: w_e =
            # zex_e * [zex_e >= m2] / (1 + m2)
            ge = rsm.tile([P, TOB, E], F32, tag="ge")
            nc.vector.tensor_tensor(
                ge, zex, m2[:, :, None].to_broadcast((P, TOB, E)), ALU.is_ge
            )
            s1 = rsm.tile([P, TOB], F32, tag="s1")
            nc.vector.tensor_scalar_add(s1, m2, 1.0)
            rcp = rsm.tile([P, TOB], F32, tag="rcp")
            nc.vector.reciprocal(rcp, s1)
            cw = rsm.tile([P, TOB, E], F32, tag="cw")
            nc.vector.tensor_tensor(cw, zex, ge, ALU.mult)
            cwn = rsm.tile([P, TOB, E], F32, tag="cwn")
            nc.vector.tensor_tensor(
                cwn, cw, rcp[:, :, None].to_broadcast((P, TOB, E)), ALU.mult
            )
            # store [TOWN*E, 1] with flat index (tb*128 + p)*8 + e
            nc.sync.dma_start(
                comb_dram[:].rearrange(
                    "(b p e) one -> p b (e one)", p=P, b=TOB, e=E
                ),
                cwn,
            )

            # prefetch combine-weight rows (router output, ready long ago)
            # before the gpsimd queue blocks on the collective
            cbs = []
            for sidx in range(NCON * TOB):
                cb = rgp.tile([P, 1], F32, tag=f"cb{sidx}", name=f"cb{sidx}")
                nc.gpsimd.indirect_dma_start(
                    out=cb[:],
                    out_offset=None,
                    in_=comb_dram[:],
                    in_offset=bass.IndirectOffsetOnAxis(
                        ap=cbi_sb[:, sidx:sidx + 1], axis=0
                    ),
                    bounds_check=TOWN * E - 1,
                    oob_is_err=False,
                )
                cbs.append(cb)
            # ============ mm1 base: own 256 tokens, full I =================
            xtO = xtp.tile([P, KO, TOWN], F16, tag="xt", name="xtO")
            nc.sync.dma_start(xtO, xtO_d[:])
            ht_b = htp.tile([P, IC_B, TOWN], F16, tag="hte", name="ht_b")
            for j in range(IC_B):
                slab = wgup.tile([P, KO, 2 * P], F16, tag="slab", name=f"slb{j}")
                slab_dma = nc.sync.dma_start(slab, wgu_d[:, 2 * SLE + j])
                if j == IC_B - 1:
                    last_mm1b_slab_dma = slab_dma
                pg = ps1.tile([P, TOWN], F32, tag="pg", name=f"bpg{j}")
                pu = ps1.tile([P, TOWN], F32, tag="pu", name=f"bpu{j}")
                # interleave gate/up so each LDWEIGHTS hides under the
                # previous matmul (N=256 leaves no slack otherwise)
                for k in range(KO):
                    nc.tensor.matmul(
                        pg, slab[:, k, 0:P], xtO[:, k, :],
                        start=(k == 0), stop=(k == KO - 1),
                    )
                    nc.tensor.matmul(
                        pu, slab[:, k, P:2 * P], xtO[:, k, :],
                        start=(k == 0), stop=(k == KO - 1),
                    )
                sil = tmpp.tile([P, TOWN], F16, tag="sil")
                nc.scalar.activation(sil, pg, AF.Silu)
                nc.vector.tensor_tensor(ht_b[:, j, :], sil, pu, ALU.mult)
                if j == 22:
                    # prefetch mm2b's first column chunk of base down weights
                    # here: emitted later they would queue behind the rest of
                    # the slab DMAs and stall mm2b's first matmuls
                    wdb0 = []
                    for ss in range(IC_B // (SLE // 2)):
                        w = wdp.tile(
                            [P, SLE // 2, HN], F16, tag="wsl", name=f"wb0_{ss}"
                        )
                        nc.sync.dma_start(
                            w,
                            wdb_d[:, 0, ss * (SLE // 2):(ss + 1) * (SLE // 2)],
                        )
                        wdb0.append(w)

            # ===== receive: gather 4 partial rows per token, accumulate ====
            # The weighted expert accumulation has no base dependency, so it
            # runs on the vector engine underneath the base down-projection.
            accs = [
                osbp.tile([P, H], F16, tag=f"osb{tb_}", name=f"osb{tb_}")
                for tb_ in range(TOB)
            ]
            for q in range(HNC):
                qsl = slice(q * HN, (q + 1) * HN)
                for tb in range(TOB):
                    for j in range(NCON):
                        sidx = j * TOB + tb
                        rg = rgp.tile([P, HN], F16, tag="rg")
                        rg_dma = nc.gpsimd.indirect_dma_start(
                            out=rg[:],
                            out_offset=None,
                            in_=recv_dram[q][:],
                            in_offset=bass.IndirectOffsetOnAxis(
                                ap=rvi_sb[:, sidx:sidx + 1], axis=0
                            ),
                            bounds_check=NCORE * SLOT - 1,
                            oob_is_err=False,
                        )
                        # Pin behind mm1b's last weight load: issued earlier,
                        # this gather's A2A-completion wait head-of-line
                        # blocks later DMAs sharing its completion lane.
                        bass._add_dep_helper(
                            rg_dma.ins, last_mm1b_slab_dma.ins, sync=True,
                            reason="defer recv gather",
                        )
                        if j == 0:
                            nc.vector.tensor_scalar_mul(
                                accs[tb][:, qsl], rg[:], cbs[sidx][:]
                            )
                        else:
                            nc.vector.scalar_tensor_tensor(
                                accs[tb][:, qsl], rg[:], cbs[sidx][:],
                                accs[tb][:, qsl], ALU.mult, ALU.add,
                            )

            # ============ mm2 base (down) on own tokens ====================
            base_lo = yesp.tile([P, TOB, HF], F16, tag="yes", name="base_lo")
            base_hi = xk32p.tile([P, TOB, HF], F16, tag="xk", name="base_hi")
            for cc in range(HNC):
                nsub = IC_B // (SLE // 2)
                if cc == 0:
                    bsubs = wdb0
                else:
                    bsubs = []
                    for ss in range(nsub):
                        w = wdp.tile(
                            [P, SLE // 2, HN], F16, tag="wsl", name=f"wb{cc}_{ss}"
                        )
                        nc.sync.dma_start(
                            w, wdb_d[:, cc, ss * (SLE // 2):(ss + 1) * (SLE // 2)]
                        )
                        bsubs.append(w)
                for tb in range(TOB):
                    py = ps2.tile([P, HN], F32, tag="py", name=f"pyb{cc}_{tb}")
                    for j in range(IC_B):
                        nc.tensor.matmul(
                            py, ht_b[:, j, tb * P:(tb + 1) * P],
                            bsubs[j // (SLE // 2)][:, j % (SLE // 2), :],
                            start=(j == 0), stop=(j == IC_B - 1),
                        )
                    bdst = base_lo if cc < HNC // 2 else base_hi
                    bcc = cc % (HNC // 2)
                    nc.scalar.activation(
                        bdst[:, tb, bcc * HN:(bcc + 1) * HN], py, AF.Copy
                    )

            # ====== add base rows; low half first so it hides under the ====
            # ====== remaining base down-projection; write output shard =====
            for half in range(2):
                hsl = slice(half * HF, (half + 1) * HF)
                base_h = (base_lo, base_hi)[half]
                for tb in range(TOB):
                    nc.vector.tensor_tensor(
                        accs[tb][:, hsl], accs[tb][:, hsl], base_h[:, tb, :],
                        ALU.add,
                    )
            for tb in range(TOB):
                nc.sync.dma_start(out_d[tb], accs[tb])

    return nc


def _prep_inputs(x, gate_w, base_gate_up, base_down, expert_gate_up, expert_down):
    xf = np.ascontiguousarray(np.asarray(x, np.float32).reshape(NTOK, H))
    xT = np.ascontiguousarray(xf.reshape(NTOK, KO, P).transpose(2, 1, 0))
    xt16 = xT.astype(np.float16)
    gwf = np.asarray(gate_w, np.float32)
    gwp = np.ascontiguousarray(gwf.reshape(KO, P, E).transpose(1, 0, 2))

    # host-side dispatch: which tokens go to which expert (top-2 of logits)
    logits = xf @ gwf
    order = np.argsort(-logits, axis=1)
    top2 = order[:, :2]
    sel = [np.where((top2 == c).any(axis=1))[0].astype(np.int64) for c in range(E)]
    counts = np.array([len(s) for s in sel])

    # pair heavy experts with light ones so pair token sums are near-equal;
    # cores 2p / 2p+1 hold the low / high I-halves of pair p's two experts
    od = np.argsort(-counts, kind="stable")
    pairs = [(int(od[i]), int(od[E - 1 - i])) for i in range(NPAIR)]
    e2pr = {}
    for pi, (a, b) in enumerate(pairs):
        e2pr[a] = (pi, 0)
        e2pr[b] = (pi, 1)

    CA = int(max(counts[a] for a, b in pairs))
    CB = int(max(counts[b] for a, b in pairs))
    NTCA = (CA + P - 1) // P
    NTCB = (CB + P - 1) // P
    CAp, CBp = NTCA * P, NTCB * P
    NTC = NTCA + NTCB

    # per-pair concatenated token columns: [a tokens|pad] + [b tokens|pad];
    # send position = order of appearance within the (pair -> home) group
    pair_flat = []
    pair_pos = []
    pair_colof = []
    max_grp = 0
    for pi, (a, b) in enumerate(pairs):
        La, Lb = sel[a], sel[b]
        flat = np.full(CAp + CBp, -1, np.int64)
        flat[: len(La)] = La
        flat[CAp:CAp + len(Lb)] = Lb
        real = np.nonzero(flat >= 0)[0]
        pos = np.full(CAp + CBp, OOB_IDX, np.int64)
        cnt = np.zeros(NCORE, np.int64)
        colof = {}
        for ci in real:
            hm = flat[ci] // TOWN
            pos[ci] = cnt[hm]
            cnt[hm] += 1
            colof[(int(flat[ci]), 0 if ci < CAp else 1)] = ci
        max_grp = max(max_grp, int(cnt.max()))
        pair_flat.append(flat)
        pair_pos.append(pos)
        pair_colof.append(colof)
    SLOT = (max_grp + 3) // 4 * 4

    # per-pair send index: column (block bi, partition p) -> home*SLOT + pos
    dsti_p = []
    for pi in range(NPAIR):
        flat, pos = pair_flat[pi], pair_pos[pi]
        dst = np.where(
            flat >= 0, (flat // TOWN) * SLOT + pos, OOB_IDX
        ).astype(np.int64)
        dsti_p.append(
            np.ascontiguousarray(dst.reshape(NTC, P).T.astype(np.int32))
        )

    # per-core receive index: own token t, contribution j in 0..3 =
    # (expert rank j//2 sorted, I-half j%2): recv row = sender*SLOT + pos
    rvi = np.zeros((NCORE, P, NCON * TOB), np.int32)
    cbi = np.zeros((NCORE, P, NCON * TOB), np.int32)
    for hme in range(NCORE):
        for tl in range(TOWN):
            t = hme * TOWN + tl
            tb, p = divmod(tl, P)
            es = np.sort(top2[t])
            for j2, e in enumerate(es):
                pi, role = e2pr[int(e)]
                ci = pair_colof[pi][(t, role)]
                ps_ = int(pair_pos[pi][ci])
                for half in range(2):
                    j = j2 * 2 + half
                    sender = 2 * pi + half
                    rvi[hme, p, j * TOB + tb] = sender * SLOT + ps_
                    cbi[hme, p, j * TOB + tb] = tl * E + e

    bgu = np.asarray(base_gate_up, np.float32)
    gb_ = bgu[:, :I_EXP].reshape(H, IC_B, P)
    ub_ = bgu[:, I_EXP:].reshape(H, IC_B, P)
    pb_ = np.concatenate([gb_, ub_], axis=2)  # [H, IC_B, 2P]
    bd = np.asarray(base_down, np.float32)
    wdb_p = np.ascontiguousarray(
        bd.reshape(IC_B, P, HNC, HN).transpose(1, 2, 0, 3)
    ).astype(np.float16)

    in_maps = []
    for c in range(NCORE):
        pi, half = divmod(c, 2)
        a, b = pairs[pi]

        def half_slabs(We):
            We = np.asarray(We, np.float32)
            g = We[:, half * HI:(half + 1) * HI].reshape(H, SLE, P)
            u = We[:, I_EXP + half * HI:I_EXP + (half + 1) * HI].reshape(
                H, SLE, P
            )
            return np.concatenate([g, u], axis=2)  # [H, SLE, 2P]

        allp = np.concatenate(
            [half_slabs(expert_gate_up[a]), half_slabs(expert_gate_up[b]), pb_],
            axis=1,
        )  # [H, ICT, 2P]
        wgu_p = np.ascontiguousarray(
            allp.reshape(KO, P, ICT, 2 * P).transpose(1, 2, 0, 3)
        ).astype(np.float16)

        def half_down(ed_):
            d = np.asarray(ed_, np.float32)[half * HI:(half + 1) * HI]
            return d.reshape(SLE, P, HNC, HN).transpose(1, 2, 0, 3)

        wde_p = np.ascontiguousarray(
            np.stack(
                [half_down(expert_down[a]), half_down(expert_down[b])], axis=1
            )
        ).astype(np.float16)  # [P, 2, HNC, SLE, HN]

        flat = pair_flat[pi]
        real = flat >= 0
        xe = np.zeros((P, KO, CAp + CBp), np.float16)
        xe[:, :, real] = xt16[:, :, flat[real]]
        own = slice(c * TOWN, (c + 1) * TOWN)
        in_maps.append(
            dict(
                xeT=np.ascontiguousarray(xe),
                xtO=np.ascontiguousarray(xt16[:, :, own]),
                xrO=np.ascontiguousarray(xT[:, :, own]),
                wgu=wgu_p, wde=wde_p, wdb=wdb_p, gw=gwp,
                dsti=dsti_p[pi], rvi=rvi[c], cbi=cbi[c],
            )
        )
    return in_maps, CA, CB, SLOT


LAST_RESULTS = None


def kernel(x, gate_w, base_gate_up, base_down, expert_gate_up, expert_down):
    global LAST_RESULTS
    in_maps, CA, CB, SLOT = _prep_inputs(
        x, gate_w, base_gate_up, base_down, expert_gate_up, expert_down
    )
    nc = _build(CA, CB, SLOT)
    if not nc.is_finalized():
        nc.finalize()
    res = run_bass_kernel_spmd(nc, in_maps, core_ids=list(range(NCORE)))
    LAST_RESULTS = res
    y = np.empty((NTOK, H), np.float32)
    for c in range(NCORE):
        o = res.results[c]["out"]  # [TOB, P, H] f16
        y[c * TOWN:(c + 1) * TOWN] = o.reshape(TOWN, H).astype(np.float32)
    return y.reshape(1, NTOK, H)


if __name__ == "__main__":
    nc = _build(545, 510, 160)
    print("build ok; instructions:",
          sum(len(b.instructions) for b in nc.main_func.blocks))
